# revision 1
# baseline (speedup 1.0000x reference)
"""BatchHardTripletLoss kernel for 8 Trainium2 NeuronCores.

Math (matches the jax reference):
  dist2[i,j] = |e1_i|^2 + |e2_j|^2 - 2 e1.e2 + 2*eps*(s1_i - s2_j) + D*eps^2
             = a[i] + (b[j] - 2*G[i,j])
  pos_max[i] = sqrt(clip(a[i] + max_{j in pos}(b[j] - 2 G[i,j]), 0))
  neg_min[i] = sqrt(clip(a[i] + min_{j in neg}(b[j] - 2 G[i,j]), 0))
  loss = mean over pos anchors of relu(pos_max - neg_min + margin)

Device strategy (data parallel over emb1 rows, hint-compliant):
  - Host: sort emb2 rows so target==1 rows come first (masks become
    contiguous column ranges), precompute a[i]/b[j] row stats, transpose
    both embeddings to [D=128, rows] layout, cast to bf16 (rel err of the
    final loss ~1e-5, verified), scale emb1 by -2.
  - Each core gets 1024 anchor rows: computes G-blocks on TensorE
    (bf16, K=128, N=512 per matmul into fp32 PSUM) and reduces with the
    fused DVE op tensor_tensor_reduce: accum = reduce_minmax(psum + bias)
    chained across column groups via the scalar initial value.
  - Device output per core: [128, 16] fp32 = per-i-tile max/min partials.
  - Host: adds a[i], sqrt, margin/relu, weighted mean (O(B) work).
"""

import os
import sys

for _p in ("/opt/trn_rl_repo",):
    if _p not in sys.path:
        sys.path.insert(0, _p)

import numpy as np
import ml_dtypes

EPS = 1e-6
MARGIN = 0.2
B = 8192
D = 128
NCORES = 8
SH = B // NCORES      # anchors per core
NIT = SH // 128       # i-tiles of 128 anchors per core
GW = 2048             # candidate-group width = 4 PSUM banks
NG = B // GW
PSUM_BUFS = 2
BIG = 1.0e30

_programs = {}
LAST_RESULTS = None   # BassKernelResults of the most recent run (for profiling)


def _build_program(k: int):
    """Bass program for one core; k = number of positive candidates
    (boundary between the max- and min-reduced column ranges)."""
    import concourse.bacc as bacc
    import concourse.tile as tile
    from concourse import mybir

    f32 = mybir.dt.float32
    bf16 = mybir.dt.bfloat16
    AOT = mybir.AluOpType

    nc = bacc.Bacc(None)
    e1t = nc.declare_dram_parameter("e1t", [D, SH], bf16, isOutput=False)
    e2t = nc.declare_dram_parameter("e2t", [D, B], bf16, isOutput=False)
    tailw = nc.declare_dram_parameter("tailw", [16, SH], bf16, isOutput=False)
    trhs = nc.declare_dram_parameter("trhs", [16, B], bf16, isOutput=False)
    outp = nc.declare_dram_parameter("out", [128, 2 * NIT], f32, isOutput=True)

    # per-group reduction segments: (lo, hi, is_pos) in global column coords
    def group_segs(g):
        glo, ghi = g * GW, (g + 1) * GW
        segs = []
        if glo < k:
            segs.append((glo, min(ghi, k), True))
        if ghi > k:
            segs.append((max(glo, k), ghi, False))
        return segs

    n_pos_segs = sum(1 for g in range(NG) for s in group_segs(g) if s[2])
    n_neg_segs = sum(1 for g in range(NG) for s in group_segs(g) if not s[2])

    with tile.TileContext(nc) as tc:
        with (
            tc.tile_pool(name="const", bufs=1) as cpool,
            tc.tile_pool(name="e2p", bufs=NG) as e2pool,
            tc.tile_pool(name="ps", bufs=PSUM_BUFS, space="PSUM") as pspool,
            tc.tile_pool(name="red", bufs=4) as redpool,
        ):
            e1sb = cpool.tile([D, SH], bf16, tag="e1sb")
            nc.sync.dma_start(e1sb[:], e1t[:])
            twsb = cpool.tile([128, SH], bf16, tag="twsb")
            trsb = cpool.tile([128, B], bf16, tag="trsb")
            for s in range(4):
                nc.sync.dma_start(twsb[32 * s:32 * s + 4, :], tailw[4 * s:4 * s + 4, :])
            outsb = cpool.tile([128, 2 * NIT], f32, tag="outsb")

            e2sb = []
            for g in range(NG):
                e2c = e2pool.tile([D, GW], bf16, tag="e2c")
                nc.sync.dma_start(e2c[:], e2t[:, g * GW:(g + 1) * GW])
                e2sb.append(e2c)
                if g == 0:
                    for s in range(4):
                        nc.sync.dma_start(
                            trsb[32 * s:32 * s + 4, :], trhs[4 * s:4 * s + 4, :]
                        )

            for it in range(NIT):
                icols = slice(it * 128, (it + 1) * 128)
                w = e1sb[0:126, icols]
                posb = redpool.tile([128, n_pos_segs], f32, tag="posb")
                negb = redpool.tile([128, n_neg_segs], f32, tag="negb")
                ip = 0
                ineg = 0
                for g in range(NG):
                    ps = pspool.tile([128, GW], f32, tag="ps")
                    # K=126 mains (embedding dims 0..125)
                    for s in range(GW // 512):
                        nc.tensor.matmul(
                            ps[:, s * 512:(s + 1) * 512],
                            w,
                            e2sb[g][0:126, s * 512:(s + 1) * 512],
                            start=True,
                            stop=False,
                        )
                    # K=4 tails (dims 126,127 + bias hi/lo), 4-way
                    # row-packed so the four sub-tiles run concurrently
                    for s in range(GW // 512):
                        j0 = g * GW + s * 512
                        nc.tensor.matmul(
                            ps[:, s * 512:(s + 1) * 512],
                            twsb[32 * s:32 * s + 4, icols],
                            trsb[32 * s:32 * s + 4, j0:j0 + 512],
                            start=False,
                            stop=True,
                            tile_position=(32 * s, 0),
                        )
                    for lo, hi, is_pos in group_segs(g):
                        if is_pos:
                            dst = posb[:, ip:ip + 1]
                            ip += 1
                        else:
                            dst = negb[:, ineg:ineg + 1]
                            ineg += 1
                        nc.vector.tensor_reduce(
                            dst,
                            ps[:, lo - g * GW:hi - g * GW],
                            axis=mybir.AxisListType.X,
                            op=AOT.max if is_pos else AOT.min,
                        )
                nc.vector.tensor_reduce(
                    outsb[:, it:it + 1], posb[:],
                    axis=mybir.AxisListType.X, op=AOT.max,
                )
                nc.vector.tensor_reduce(
                    outsb[:, NIT + it:NIT + it + 1], negb[:],
                    axis=mybir.AxisListType.X, op=AOT.min,
                )
            nc.sync.dma_start(outp[:], outsb[:])
    nc.compile()
    return nc


def _host_prep(emb1, emb2, target):
    tpos = target == 1
    k = int(tpos.sum())
    perm = np.concatenate([np.nonzero(tpos)[0], np.nonzero(~tpos)[0]])
    e2s = emb2[perm]
    e2d = e2s.astype(np.float64)
    e1d = emb1.astype(np.float64)
    b = (e2d * e2d).sum(1) - (2.0 * EPS) * e2d.sum(1)
    a = (e1d * e1d).sum(1) + (2.0 * EPS) * e1d.sum(1) + D * EPS * EPS
    e1tb = np.ascontiguousarray((-2.0 * emb1).T.astype(ml_dtypes.bfloat16))
    e2tb = np.ascontiguousarray(e2s.T.astype(ml_dtypes.bfloat16))
    bhi = b.astype(np.float32).astype(ml_dtypes.bfloat16)
    blo = (b.astype(np.float32) - bhi.astype(np.float32)).astype(ml_dtypes.bfloat16)
    # K=4 tail operands; on device row 4s+r lands at partition 32s+r so the
    # four 512-wide sub-tiles of a group can row-pack on the PE array.
    tailw = np.zeros((16, B), dtype=ml_dtypes.bfloat16)
    trhs = np.zeros((16, B), dtype=ml_dtypes.bfloat16)
    one = np.ones(B, dtype=ml_dtypes.bfloat16)
    for s in range(4):
        tailw[4 * s + 0] = e1tb[126]
        tailw[4 * s + 1] = e1tb[127]
        tailw[4 * s + 2] = one
        tailw[4 * s + 3] = one
        trhs[4 * s + 0] = e2tb[126]
        trhs[4 * s + 1] = e2tb[127]
        trhs[4 * s + 2] = bhi
        trhs[4 * s + 3] = blo
    return k, a, e1tb, e2tb, tailw, trhs, tpos


def _host_finish(a, Mp, mn, tpos, k):
    pos2 = np.clip(a + Mp.astype(np.float64), 0.0, None)
    neg2 = np.clip(a + mn.astype(np.float64), 0.0, None)
    per = np.clip(np.sqrt(pos2) - np.sqrt(neg2) + MARGIN, 0.0, None)
    return np.float32((per * tpos).sum() / k)


def _numpy_fallback(emb1, emb2, target):
    # exact reference recomputation in numpy (degenerate target mixes)
    e1 = emb1.astype(np.float64)
    e2 = emb2.astype(np.float64)
    sq = (
        (e1 * e1).sum(1)[:, None]
        + (e2 * e2).sum(1)[None, :]
        - 2.0 * (e1 @ e2.T)
        + 2.0 * EPS * (e1.sum(1)[:, None] - e2.sum(1)[None, :])
        + D * EPS * EPS
    )
    dist = np.sqrt(np.clip(sq, 0.0, None))
    pos = target == 1
    neg = target == 0
    pos_max = np.where(pos[None, :], dist, -np.inf).max(1)
    neg_min = np.where(neg[None, :], dist, np.inf).min(1)
    per = np.maximum(pos_max - neg_min + MARGIN, 0.0)
    w = pos.astype(np.float64)
    return np.float32((per * w).sum() / w.sum())


def kernel(emb1, emb2, target):
    global LAST_RESULTS
    emb1 = np.asarray(emb1, dtype=np.float32)
    emb2 = np.asarray(emb2, dtype=np.float32)
    target = np.asarray(target)
    assert emb1.shape == (B, D) and emb2.shape == (B, D)

    k = int((target == 1).sum())
    if k == 0 or k == B:
        return _numpy_fallback(emb1, emb2, target)

    k, a, e1tb, e2tb, tailw, trhs, tpos = _host_prep(emb1, emb2, target)

    nc = _programs.get(k)
    if nc is None:
        nc = _build_program(k)
        _programs[k] = nc

    from concourse.bass_utils import run_bass_kernel_spmd

    in_maps = [
        {
            "e1t": np.ascontiguousarray(e1tb[:, c * SH:(c + 1) * SH]),
            "e2t": e2tb,
            "tailw": np.ascontiguousarray(tailw[:, c * SH:(c + 1) * SH]),
            "trhs": trhs,
        }
        for c in range(NCORES)
    ]
    res = run_bass_kernel_spmd(nc, in_maps, core_ids=list(range(NCORES)))
    LAST_RESULTS = res

    Mp = np.concatenate(
        [np.asarray(res.results[c]["out"])[:, :NIT].T.reshape(-1) for c in range(NCORES)]
    )
    mn = np.concatenate(
        [np.asarray(res.results[c]["out"])[:, NIT:].T.reshape(-1) for c in range(NCORES)]
    )
    return _host_finish(a, Mp, mn, tpos, k)



# revision 4
# speedup vs baseline: 1.0031x; 1.0031x over previous
"""BatchHardTripletLoss kernel for 8 Trainium2 NeuronCores.

Math (matches the jax reference):
  dist2[i,j] = |e1_i|^2 + |e2_j|^2 - 2 e1.e2 + 2*eps*(s1_i - s2_j) + D*eps^2
             = a[i] + (b[j] - 2*G[i,j])
  pos_max[i] = sqrt(clip(a[i] + max_{j in pos}(b[j] - 2 G[i,j]), 0))
  neg_min[i] = sqrt(clip(a[i] + min_{j in neg}(b[j] - 2 G[i,j]), 0))
  loss = mean over pos anchors of relu(pos_max - neg_min + margin)

Device strategy (data parallel over emb1 rows, hint-compliant):
  - Host: sort emb2 rows so target==1 rows come first (masks become
    contiguous column ranges), precompute a[i]/b[j] row stats, transpose
    both embeddings to [D=128, rows] layout, cast to bf16 (rel err of the
    final loss ~1e-5, verified), scale emb1 by -2.
  - Each core gets 1024 anchor rows: computes G-blocks on TensorE
    (bf16, K=128, N=512 per matmul into fp32 PSUM) and reduces with the
    fused DVE op tensor_tensor_reduce: accum = reduce_minmax(psum + bias)
    chained across column groups via the scalar initial value.
  - Device output per core: [128, 16] fp32 = per-i-tile max/min partials.
  - Host: adds a[i], sqrt, margin/relu, weighted mean (O(B) work).
"""

import os
import sys

for _p in ("/opt/trn_rl_repo",):
    if _p not in sys.path:
        sys.path.insert(0, _p)

import numpy as np
import ml_dtypes

EPS = 1e-6
MARGIN = 0.2
B = 8192
D = 128
NCORES = 8
SH = B // NCORES      # anchors per core
NIT = SH // 128       # i-tiles of 128 anchors per core
GW = 2048             # candidate-group width = 4 PSUM banks
NG = B // GW
PSUM_BUFS = 2
BIG = 1.0e30

_programs = {}
LAST_RESULTS = None   # BassKernelResults of the most recent run (for profiling)


def _build_program(k: int):
    """Bass program for one core; k = number of positive candidates
    (boundary between the max- and min-reduced column ranges)."""
    import concourse.bacc as bacc
    import concourse.tile as tile
    from concourse import mybir

    f32 = mybir.dt.float32
    bf16 = mybir.dt.bfloat16
    AOT = mybir.AluOpType

    nc = bacc.Bacc(None)
    e1t = nc.declare_dram_parameter("e1t", [D, SH], bf16, isOutput=False)
    e2t = nc.declare_dram_parameter("e2t", [D, B], bf16, isOutput=False)
    tailw = nc.declare_dram_parameter("tailw", [16, SH], bf16, isOutput=False)
    trhs = nc.declare_dram_parameter("trhs", [16, B], bf16, isOutput=False)
    outp = nc.declare_dram_parameter("out", [128, 2 * NIT], f32, isOutput=True)

    # per-group reduction segments: (lo, hi, is_pos) in global column coords
    def group_segs(g):
        glo, ghi = g * GW, (g + 1) * GW
        segs = []
        if glo < k:
            segs.append((glo, min(ghi, k), True))
        if ghi > k:
            segs.append((max(glo, k), ghi, False))
        return segs

    n_pos_segs = sum(1 for g in range(NG) for s in group_segs(g) if s[2])
    n_neg_segs = sum(1 for g in range(NG) for s in group_segs(g) if not s[2])

    with tile.TileContext(nc) as tc:
        with (
            tc.tile_pool(name="const", bufs=1) as cpool,
            tc.tile_pool(name="e2p", bufs=NG) as e2pool,
            tc.tile_pool(name="ps", bufs=PSUM_BUFS, space="PSUM") as pspool,
            tc.tile_pool(name="red", bufs=4) as redpool,
        ):
            e1sb = cpool.tile([D, SH], bf16, tag="e1sb")
            nc.sync.dma_start(e1sb[:], e1t[:])
            twsb = cpool.tile([128, SH], bf16, tag="twsb")
            trsb = cpool.tile([128, B], bf16, tag="trsb")
            for s in range(4):
                nc.sync.dma_start(twsb[32 * s:32 * s + 4, :], tailw[4 * s:4 * s + 4, :])
            outsb = cpool.tile([128, 2 * NIT], f32, tag="outsb")

            e2sb = []
            for g in range(NG):
                e2c = e2pool.tile([D, GW], bf16, tag="e2c")
                nc.sync.dma_start(e2c[:], e2t[:, g * GW:(g + 1) * GW])
                e2sb.append(e2c)
                if g == 0:
                    for s in range(4):
                        nc.sync.dma_start(
                            trsb[32 * s:32 * s + 4, :], trhs[4 * s:4 * s + 4, :]
                        )

            for it in range(NIT):
                icols = slice(it * 128, (it + 1) * 128)
                w = e1sb[0:126, icols]
                posb = redpool.tile([128, n_pos_segs], f32, tag="posb")
                negb = redpool.tile([128, n_neg_segs], f32, tag="negb")
                ip = 0
                ineg = 0
                for g in range(NG):
                    ps = pspool.tile([128, GW], f32, tag="ps")
                    # K=126 mains (embedding dims 0..125)
                    for s in range(GW // 512):
                        nc.tensor.matmul(
                            ps[:, s * 512:(s + 1) * 512],
                            w,
                            e2sb[g][0:126, s * 512:(s + 1) * 512],
                            start=True,
                            stop=False,
                        )
                    # K=4 tails (dims 126,127 + bias hi/lo), 4-way
                    # row-packed so the four sub-tiles run concurrently
                    for s in range(GW // 512):
                        j0 = g * GW + s * 512
                        nc.tensor.matmul(
                            ps[:, s * 512:(s + 1) * 512],
                            twsb[32 * s:32 * s + 4, icols],
                            trsb[32 * s:32 * s + 4, j0:j0 + 512],
                            start=False,
                            stop=True,
                            tile_position=(32 * s, 0),
                        )
                    for lo, hi, is_pos in group_segs(g):
                        if is_pos:
                            dst = posb[:, ip:ip + 1]
                            ip += 1
                        else:
                            dst = negb[:, ineg:ineg + 1]
                            ineg += 1
                        nc.vector.tensor_reduce(
                            dst,
                            ps[:, lo - g * GW:hi - g * GW],
                            axis=mybir.AxisListType.X,
                            op=AOT.max if is_pos else AOT.min,
                        )
                nc.vector.tensor_reduce(
                    outsb[:, it:it + 1], posb[:],
                    axis=mybir.AxisListType.X, op=AOT.max,
                )
                nc.vector.tensor_reduce(
                    outsb[:, NIT + it:NIT + it + 1], negb[:],
                    axis=mybir.AxisListType.X, op=AOT.min,
                )
            nc.sync.dma_start(outp[:], outsb[:])
    nc.compile()
    return nc


def _host_prep(emb1, emb2, target):
    tpos = target == 1
    k = int(tpos.sum())
    perm = np.concatenate([np.nonzero(tpos)[0], np.nonzero(~tpos)[0]])
    e2s = emb2[perm]
    e2d = e2s.astype(np.float64)
    e1d = emb1.astype(np.float64)
    b = (e2d * e2d).sum(1) - (2.0 * EPS) * e2d.sum(1)
    a = (e1d * e1d).sum(1) + (2.0 * EPS) * e1d.sum(1) + D * EPS * EPS
    e1tb = np.ascontiguousarray((-2.0 * emb1).T.astype(ml_dtypes.bfloat16))
    e2tb = np.ascontiguousarray(e2s.T.astype(ml_dtypes.bfloat16))
    bhi = b.astype(np.float32).astype(ml_dtypes.bfloat16)
    blo = (b.astype(np.float32) - bhi.astype(np.float32)).astype(ml_dtypes.bfloat16)
    # K=4 tail operands; on device row 4s+r lands at partition 32s+r so the
    # four 512-wide sub-tiles of a group can row-pack on the PE array.
    tailw = np.zeros((16, B), dtype=ml_dtypes.bfloat16)
    trhs = np.zeros((16, B), dtype=ml_dtypes.bfloat16)
    one = np.ones(B, dtype=ml_dtypes.bfloat16)
    for s in range(4):
        tailw[4 * s + 0] = e1tb[126]
        tailw[4 * s + 1] = e1tb[127]
        tailw[4 * s + 2] = one
        tailw[4 * s + 3] = one
        trhs[4 * s + 0] = e2tb[126]
        trhs[4 * s + 1] = e2tb[127]
        trhs[4 * s + 2] = bhi
        trhs[4 * s + 3] = blo
    return k, a, e1tb, e2tb, tailw, trhs, tpos


def _host_finish(a, Mp, mn, tpos, k):
    pos2 = np.clip(a + Mp.astype(np.float64), 0.0, None)
    neg2 = np.clip(a + mn.astype(np.float64), 0.0, None)
    per = np.clip(np.sqrt(pos2) - np.sqrt(neg2) + MARGIN, 0.0, None)
    return np.float32((per * tpos).sum() / k)


def _numpy_fallback(emb1, emb2, target):
    # exact reference recomputation in numpy (degenerate target mixes)
    e1 = emb1.astype(np.float64)
    e2 = emb2.astype(np.float64)
    sq = (
        (e1 * e1).sum(1)[:, None]
        + (e2 * e2).sum(1)[None, :]
        - 2.0 * (e1 @ e2.T)
        + 2.0 * EPS * (e1.sum(1)[:, None] - e2.sum(1)[None, :])
        + D * EPS * EPS
    )
    dist = np.sqrt(np.clip(sq, 0.0, None))
    pos = target == 1
    neg = target == 0
    pos_max = np.where(pos[None, :], dist, -np.inf).max(1)
    neg_min = np.where(neg[None, :], dist, np.inf).min(1)
    per = np.maximum(pos_max - neg_min + MARGIN, 0.0)
    w = pos.astype(np.float64)
    return np.float32((per * w).sum() / w.sum())


def kernel(emb1, emb2, target):
    global LAST_RESULTS
    emb1 = np.asarray(emb1, dtype=np.float32)
    emb2 = np.asarray(emb2, dtype=np.float32)
    target = np.asarray(target)
    assert emb1.shape == (B, D) and emb2.shape == (B, D)

    k = int((target == 1).sum())
    if k == 0 or k == B:
        return _numpy_fallback(emb1, emb2, target)

    k, a, e1tb, e2tb, tailw, trhs, tpos = _host_prep(emb1, emb2, target)

    nc = _programs.get(k)
    if nc is None:
        nc = _build_program(k)
        _programs[k] = nc

    from concourse.bass_utils import run_bass_kernel_spmd

    in_maps = [
        {
            "e1t": np.ascontiguousarray(e1tb[:, c * SH:(c + 1) * SH]),
            "e2t": e2tb,
            "tailw": np.ascontiguousarray(tailw[:, c * SH:(c + 1) * SH]),
            "trhs": trhs,
        }
        for c in range(NCORES)
    ]
    res = run_bass_kernel_spmd(nc, in_maps, core_ids=list(range(NCORES)))
    LAST_RESULTS = res

    Mp = np.concatenate(
        [np.asarray(res.results[c]["out"])[:, :NIT].T.reshape(-1) for c in range(NCORES)]
    )
    mn = np.concatenate(
        [np.asarray(res.results[c]["out"])[:, NIT:].T.reshape(-1) for c in range(NCORES)]
    )
    return _host_finish(a, Mp, mn, tpos, k)



# revision 5
# speedup vs baseline: 1.3388x; 1.3347x over previous
"""BatchHardTripletLoss kernel for 8 Trainium2 NeuronCores.

Math (matches the jax reference):
  dist2[i,j] = |e1_i|^2 + |e2_j|^2 - 2 e1.e2 + 2*eps*(s1_i - s2_j) + D*eps^2
             = a[i] + v[i,j],   v[i,j] = b[j] - 2<e1_i, e2_j>
  pos_max[i] = sqrt(clip(a[i] + max_{j in pos} v[i,j], 0))
  neg_min[i] = sqrt(clip(a[i] + min_{j in neg} v[i,j], 0))
  loss = mean over POS anchors of relu(pos_max - neg_min + margin)

Key structural points (vs the first-cut kernel at 106.7us):
  * The loss only reads rows i with target[i]==1, so only k~B/2 anchor
    rows of the distance matrix are computed (k//1024*1024 rows on
    device, the <1024-row remainder exactly on host in f64).
  * Negating the e2 columns (and their bias) of the neg class turns
    min into max, so every reduction is a MAX and chains through the
    per-partition scalar1 operand of the stock TENSOR_SCALAR op:
      res = max(psum_chunk, chain); accum_out = max-reduce(res)
  * The PSUM drain is split across two engines: the scalar (Act)
    engine copies ~60% of each PSUM group to SBUF as bf16, which the
    DVE then reduces at the packed 2x/4x rate; the DVE reduces the
    remaining ~40% directly from PSUM in f32 (exact).  This roughly
    halves the DVE streaming time (the old bottleneck) and keeps the
    PE busy so the HAM clock gate stays un-throttled.

Device layout (data parallel over anchor rows, hint-compliant):
  - Host: sort e2 rows pos-first, sign-flip neg rows, precompute
    a[i]/b[j] row stats in f64, b in bf16 hi+lo, transpose to
    [D=128, rows] bf16, scale e1 by -2.
  - Each core: SH = NIT*128 pos anchors; per (i-tile, col-group of
    2048) the PE computes K=126 mains (4x N=512, bf16) plus a K=4
    tail (dims 126/127 + bias hi/lo) 4-way row-packed so the four
    N=512 sub-tiles run concurrently on the PE quadrants.
  - Output per core: [128, 2*NIT] f32 chained max partials.
  - Host: adds a[i], sqrt, margin/relu, mean (O(B) work) + exact
    f64 remainder rows.
"""

import os
import sys

for _p in ("/opt/trn_rl_repo",):
    if _p not in sys.path:
        sys.path.insert(0, _p)

import numpy as np
import ml_dtypes

EPS = 1e-6
MARGIN = 0.2
B = 8192
D = 128
NCORES = 8
GW = 2048             # candidate-group width = 4 PSUM banks
NG = B // GW
PSUM_BUFS = 2
BIG = 1.0e30
CFRAC = 0.60          # fraction of each segment Act-copied to bf16 for 4x DVE
MIN_SPLIT = 64        # segments narrower than this reduce directly from PSUM

_programs = {}
LAST_RESULTS = None   # BassKernelResults of the most recent run (for profiling)


def _build_program(n_it: int, k: int):
    """Bass program for one core.

    n_it: i-tiles (of 128 anchors) per core.
    k: number of positive candidate columns (boundary between the pos
       range [0,k) and the sign-flipped neg range [k,B); both classes
       reduce with MAX on device).
    """
    import concourse.bacc as bacc
    import concourse.tile as tile
    from concourse import mybir

    f32 = mybir.dt.float32
    bf16 = mybir.dt.bfloat16
    AOT = mybir.AluOpType

    SH = n_it * 128

    nc = bacc.Bacc(None)
    e1t = nc.declare_dram_parameter("e1t", [D, SH], bf16, isOutput=False)
    e2t = nc.declare_dram_parameter("e2t", [D, B], bf16, isOutput=False)
    tailw = nc.declare_dram_parameter("tailw", [16, SH], bf16, isOutput=False)
    trhs = nc.declare_dram_parameter("trhs", [16, B], bf16, isOutput=False)
    outp = nc.declare_dram_parameter("out", [128, 2 * n_it], f32, isOutput=True)

    # per-group reduction segments: (lo, hi, is_pos) in global column coords
    def group_segs(g):
        glo, ghi = g * GW, (g + 1) * GW
        segs = []
        if glo < k:
            segs.append((glo, min(ghi, k), True))
        if ghi > k:
            segs.append((max(glo, k), ghi, False))
        return segs

    with tile.TileContext(nc) as tc:
        with (
            tc.tile_pool(name="const", bufs=1) as cpool,
            tc.tile_pool(name="e2p", bufs=NG) as e2pool,
            tc.tile_pool(name="ps", bufs=PSUM_BUFS, space="PSUM") as pspool,
            tc.tile_pool(name="cp", bufs=3) as cppool,
            tc.tile_pool(name="red", bufs=2) as redpool,
        ):
            e1sb = cpool.tile([D, SH], bf16, tag="e1sb")
            nc.sync.dma_start(e1sb[:], e1t[:])
            twsb = cpool.tile([128, SH], bf16, tag="twsb")
            trsb = cpool.tile([128, B], bf16, tag="trsb")
            for s in range(4):
                nc.sync.dma_start(twsb[32 * s:32 * s + 4, :], tailw[4 * s:4 * s + 4, :])
            outsb = cpool.tile([128, 2 * n_it], f32, tag="outsb")
            trf = cpool.tile([128, 1024], f32, tag="trf")
            trb = cpool.tile([128, 2048], bf16, tag="trb")

            e2sb = []
            for g in range(NG):
                e2c = e2pool.tile([D, GW], bf16, tag="e2c")
                nc.sync.dma_start(e2c[:], e2t[:, g * GW:(g + 1) * GW])
                e2sb.append(e2c)
                if g == 0:
                    for s in range(4):
                        nc.sync.dma_start(
                            trsb[32 * s:32 * s + 4, :], trhs[4 * s:4 * s + 4, :]
                        )

            for it in range(n_it):
                icols = slice(it * 128, (it + 1) * 128)
                w = e1sb[0:126, icols]
                chain = redpool.tile([128, 16], f32, tag="chain")
                nlinks = {True: 0, False: 8}   # pos links cols 0..7, neg 8..15
                prev_acc = {True: None, False: None}
                segs_left = {
                    True: sum(1 for g in range(NG) for s in group_segs(g) if s[2]),
                    False: sum(1 for g in range(NG) for s in group_segs(g) if not s[2]),
                }

                for g in range(NG):
                    ps = pspool.tile([128, GW], f32, tag="ps")
                    # K=126 mains (embedding dims 0..125)
                    for s in range(GW // 512):
                        nc.tensor.matmul(
                            ps[:, s * 512:(s + 1) * 512],
                            w,
                            e2sb[g][0:126, s * 512:(s + 1) * 512],
                            start=True,
                            stop=False,
                        )
                    # K=4 tails (dims 126,127 + bias hi/lo), 4-way
                    # row-packed so the four sub-tiles run concurrently
                    for s in range(GW // 512):
                        j0 = g * GW + s * 512
                        nc.tensor.matmul(
                            ps[:, s * 512:(s + 1) * 512],
                            twsb[32 * s:32 * s + 4, icols],
                            trsb[32 * s:32 * s + 4, j0:j0 + 512],
                            start=False,
                            stop=True,
                            tile_position=(32 * s, 0),
                        )

                    # drain this group's segments with chained max reduces
                    for lo, hi, is_pos in group_segs(g):
                        ll, lh = lo - g * GW, hi - g * GW   # local col range
                        wseg = lh - ll
                        segs_left[is_pos] -= 1
                        final = segs_left[is_pos] == 0
                        if wseg >= MIN_SPLIT:
                            c = (int(wseg * CFRAC) // 4) * 4
                        else:
                            c = 0
                        d = wseg - c

                        def next_acc():
                            j = nlinks[is_pos]
                            nlinks[is_pos] += 1
                            return chain[:, j:j + 1]

                        def fin_acc():
                            col = (0 if is_pos else n_it) + it
                            return outsb[:, col:col + 1]

                        init = prev_acc[is_pos]
                        if c > 0:
                            cpb = cppool.tile([128, 2048], bf16, tag="cpb")
                            # Act engine: copy+cast tail part of the segment
                            nc.scalar.copy(cpb[:, 0:c], ps[:, ll + d:lh])
                        if d > 0:
                            acc = fin_acc() if (final and c == 0) else next_acc()
                            nc.vector.tensor_scalar(
                                out=trf[:, 0:d],
                                in0=ps[:, ll:ll + d],
                                scalar1=init if init is not None else -BIG,
                                scalar2=None,
                                op0=AOT.max,
                                op1=AOT.max,
                                accum_out=acc,
                            )
                            init = acc
                        if c > 0:
                            acc = fin_acc() if final else next_acc()
                            nc.vector.tensor_scalar(
                                out=trb[:, 0:c],
                                in0=cpb[:, 0:c],
                                scalar1=init if init is not None else -BIG,
                                scalar2=None,
                                op0=AOT.max,
                                op1=AOT.max,
                                accum_out=acc,
                            )
                            init = acc
                        prev_acc[is_pos] = init
            nc.sync.dma_start(outp[:], outsb[:])
    nc.compile()
    return nc


def _host_prep(emb1, emb2, target):
    tpos = target == 1
    k = int(tpos.sum())
    perm = np.concatenate([np.nonzero(tpos)[0], np.nonzero(~tpos)[0]])
    e2s = emb2[perm]
    e2d = e2s.astype(np.float64)
    b = (e2d * e2d).sum(1) - (2.0 * EPS) * e2d.sum(1)
    sgn = np.ones(B, dtype=np.float64)
    sgn[k:] = -1.0
    bsig = (b * sgn).astype(np.float32)
    e2sig = e2s * sgn[:, None].astype(np.float32)

    e1p = emb1[tpos]                       # [k, D] pos anchors
    e1d = e1p.astype(np.float64)
    a = (e1d * e1d).sum(1) + (2.0 * EPS) * e1d.sum(1) + D * EPS * EPS

    n_it = min(k // 1024, 8)
    ndev = n_it * 1024
    e1dev = e1p[:ndev]

    e1tb = np.ascontiguousarray((-2.0 * e1dev).T.astype(ml_dtypes.bfloat16))
    e2tb = np.ascontiguousarray(e2sig.T.astype(ml_dtypes.bfloat16))
    bhi = bsig.astype(ml_dtypes.bfloat16)
    blo = (bsig - bhi.astype(np.float32)).astype(ml_dtypes.bfloat16)
    # K=4 tail operands; on device row 4s+r lands at partition 32s+r so the
    # four 512-wide sub-tiles of a group can row-pack on the PE array.
    tailw = np.zeros((16, ndev), dtype=ml_dtypes.bfloat16)
    trhs = np.zeros((16, B), dtype=ml_dtypes.bfloat16)
    one = np.ones(ndev, dtype=ml_dtypes.bfloat16)
    for s in range(4):
        tailw[4 * s + 0] = e1tb[126]
        tailw[4 * s + 1] = e1tb[127]
        tailw[4 * s + 2] = one
        tailw[4 * s + 3] = one
        trhs[4 * s + 0] = e2tb[126]
        trhs[4 * s + 1] = e2tb[127]
        trhs[4 * s + 2] = bhi
        trhs[4 * s + 3] = blo
    return k, n_it, a, e1p, e1tb, e2tb, tailw, trhs


def _host_remainder(e1rem, emb2, target):
    """Exact f64 pos_max/neg_min contribution of the remainder anchors."""
    e1d = e1rem.astype(np.float64)
    e2d = emb2.astype(np.float64)
    sq = (
        (e1d * e1d).sum(1)[:, None]
        + (e2d * e2d).sum(1)[None, :]
        - 2.0 * (e1d @ e2d.T)
        + 2.0 * EPS * (e1d.sum(1)[:, None] - e2d.sum(1)[None, :])
        + D * EPS * EPS
    )
    dist = np.sqrt(np.clip(sq, 0.0, None))
    pos = target == 1
    pos_max = np.where(pos[None, :], dist, -np.inf).max(1)
    neg_min = np.where(~pos[None, :], dist, np.inf).min(1)
    return np.clip(pos_max - neg_min + MARGIN, 0.0, None).sum()


def _numpy_fallback(emb1, emb2, target):
    # exact reference recomputation in numpy (degenerate target mixes)
    e1 = emb1.astype(np.float64)
    e2 = emb2.astype(np.float64)
    sq = (
        (e1 * e1).sum(1)[:, None]
        + (e2 * e2).sum(1)[None, :]
        - 2.0 * (e1 @ e2.T)
        + 2.0 * EPS * (e1.sum(1)[:, None] - e2.sum(1)[None, :])
        + D * EPS * EPS
    )
    dist = np.sqrt(np.clip(sq, 0.0, None))
    pos = target == 1
    neg = target == 0
    pos_max = np.where(pos[None, :], dist, -np.inf).max(1)
    neg_min = np.where(neg[None, :], dist, np.inf).min(1)
    per = np.maximum(pos_max - neg_min + MARGIN, 0.0)
    w = pos.astype(np.float64)
    return np.float32((per * w).sum() / w.sum())


def kernel(emb1, emb2, target):
    global LAST_RESULTS
    emb1 = np.asarray(emb1, dtype=np.float32)
    emb2 = np.asarray(emb2, dtype=np.float32)
    target = np.asarray(target)
    assert emb1.shape == (B, D) and emb2.shape == (B, D)

    k = int((target == 1).sum())
    if k < 1024 or k == B:
        return _numpy_fallback(emb1, emb2, target)

    k, n_it, a, e1p, e1tb, e2tb, tailw, trhs = _host_prep(emb1, emb2, target)
    ndev = n_it * 1024
    SH = n_it * 128

    nc = _programs.get((n_it, k))
    if nc is None:
        nc = _build_program(n_it, k)
        _programs[(n_it, k)] = nc

    from concourse.bass_utils import run_bass_kernel_spmd

    in_maps = [
        {
            "e1t": np.ascontiguousarray(e1tb[:, c * SH:(c + 1) * SH]),
            "e2t": e2tb,
            "tailw": np.ascontiguousarray(tailw[:, c * SH:(c + 1) * SH]),
            "trhs": trhs,
        }
        for c in range(NCORES)
    ]
    res = run_bass_kernel_spmd(nc, in_maps, core_ids=list(range(NCORES)))
    LAST_RESULTS = res

    Mp = np.concatenate(
        [np.asarray(res.results[c]["out"])[:, :n_it].T.reshape(-1) for c in range(NCORES)]
    )
    Mn = np.concatenate(
        [np.asarray(res.results[c]["out"])[:, n_it:].T.reshape(-1) for c in range(NCORES)]
    )

    adev = a[:ndev]
    pos2 = np.clip(adev + Mp.astype(np.float64), 0.0, None)
    neg2 = np.clip(adev - Mn.astype(np.float64), 0.0, None)  # min v = -max(-v)
    per = np.clip(np.sqrt(pos2) - np.sqrt(neg2) + MARGIN, 0.0, None)
    total = per.sum()
    if ndev < k:
        total += _host_remainder(e1p[ndev:], emb2, target)
    return np.float32(total / k)


# revision 8
# speedup vs baseline: 1.5378x; 1.1486x over previous
"""BatchHardTripletLoss kernel for 8 Trainium2 NeuronCores.

Math (matches the jax reference):
  dist2[i,j] = |e1_i|^2 + |e2_j|^2 - 2 e1.e2 + 2*eps*(s1_i - s2_j) + D*eps^2
             = a[i] + v[i,j],   v[i,j] = b[j] - 2<e1_i, e2_j>
  pos_max[i] = sqrt(clip(a[i] + max_{j in pos} v[i,j], 0))
  neg_min[i] = sqrt(clip(a[i] + min_{j in neg} v[i,j], 0))
  loss = mean over POS anchors of relu(pos_max - neg_min + margin)

Key structural points (vs the 106.7us first-cut kernel):
  * Only rows with target[i]==1 are computed (the loss ignores the
    rest): k//1024*1024 rows on device, the remainder exactly on host.
  * Neg-class e2 columns (and their bias) are sign-flipped so both
    classes are MAX reductions.
  * Mains run in fp8 (e4m3): the PE streams 2 columns/cycle, so a
    512-col matmul issues every ~216ns even at the cold (K=4/8 HAM)
    clock.  The K=4 bf16 tails carry dims 126/127 + the bias in
    bf16 hi+lo.  Verified end-to-end fp8 rel err ~6.6e-4 (tol 2e-2).
  * The bias TAILS run FIRST (start=True) and the mains LAST: the
    drain of a group unblocks as soon as its mains finish, which
    overlaps the scheduler's weight-batched [tails A,B][mains A,B]
    PE order instead of serializing PE-phase -> drain-phase.
  * Drain: the Act engine copies each PSUM group to SBUF bf16
    (1 elem/cyc); the DVE folds the copies in-place into a
    [128,1024] bf16 accumulator per (i-tile, class) with stock
    TENSOR_TENSOR max ops at the packed 2x rate, then one 1x
    TENSOR_SCALAR accumulate-max per (i-tile, class) reduces the
    accumulator into the output, chained with any direct-reduced
    boundary slivers via its per-partition scalar operand.

Host: pos-first column sort, f64 row stats, bf16 hi/lo bias split,
transposes, fp8/bf16 casts, final sqrt/margin/mean + exact f64
remainder rows.
"""

import os
import sys

for _p in ("/opt/trn_rl_repo",):
    if _p not in sys.path:
        sys.path.insert(0, _p)

import numpy as np
import ml_dtypes

EPS = 1e-6
MARGIN = 0.2
B = 8192
D = 128
NCORES = 8
GW = 2048             # candidate-group width = 4 PSUM banks
NG = B // GW
PSUM_BUFS = 2
BIG = 1.0e30
MIN_COPY = 512        # segments narrower than this reduce directly from PSUM
ACCW = 1024           # bf16 accumulator width per (i-tile, class)

_programs = {}
LAST_RESULTS = None   # BassKernelResults of the most recent run (for profiling)


def _build_program(n_it: int, k: int):
    """Bass program for one core.

    n_it: i-tiles (of 128 anchors) per core.
    k: number of positive candidate columns (boundary between the pos
       range [0,k) and the sign-flipped neg range [k,B)).
    """
    import concourse.bacc as bacc
    import concourse.tile as tile
    from concourse import mybir

    f32 = mybir.dt.float32
    bf16 = mybir.dt.bfloat16
    fp8 = mybir.dt.float8e4
    AOT = mybir.AluOpType

    SH = n_it * 128

    nc = bacc.Bacc(None)
    e1t = nc.declare_dram_parameter("e1t", [D, SH], fp8, isOutput=False)
    e2t = nc.declare_dram_parameter("e2t", [D, B], fp8, isOutput=False)
    tailw = nc.declare_dram_parameter("tailw", [16, SH], bf16, isOutput=False)
    trhs = nc.declare_dram_parameter("trhs", [16, B], bf16, isOutput=False)
    outp = nc.declare_dram_parameter("out", [128, 2 * n_it], f32, isOutput=True)

    # per-group reduction segments: (lo, hi, is_pos) in global column coords
    def group_segs(g):
        glo, ghi = g * GW, (g + 1) * GW
        segs = []
        if glo < k:
            segs.append((glo, min(ghi, k), True))
        if ghi > k:
            segs.append((max(glo, k), ghi, False))
        return segs

    def even_chunks(w):
        """Split [0,w) into even-width chunks <= ACCW; odd leftover col
        is returned separately (handled by the direct f32 path)."""
        out = []
        pos = 0
        we = (w // 2) * 2
        while pos < we:
            cw = min(ACCW, we - pos)
            out.append((pos, cw))
            pos += cw
        return out, (w - we)   # chunks, n leftover cols (0 or 1)

    with tile.TileContext(nc) as tc:
        with (
            tc.tile_pool(name="const", bufs=1) as cpool,
            tc.tile_pool(name="e2p", bufs=NG) as e2pool,
            tc.tile_pool(name="ps", bufs=PSUM_BUFS, space="PSUM") as pspool,
            tc.tile_pool(name="cp", bufs=3) as cppool,
            tc.tile_pool(name="acc", bufs=4) as accpool,
            tc.tile_pool(name="red", bufs=2) as redpool,
        ):
            e1sb = cpool.tile([D, SH], fp8, tag="e1sb")
            nc.sync.dma_start(e1sb[:], e1t[:])
            twsb = cpool.tile([128, SH], bf16, tag="twsb")
            trsb = cpool.tile([128, B], bf16, tag="trsb")
            for s in range(4):
                nc.sync.dma_start(twsb[32 * s:32 * s + 4, :], tailw[4 * s:4 * s + 4, :])
            outsb = cpool.tile([128, 2 * n_it], f32, tag="outsb")
            trf = cpool.tile([128, 2048], f32, tag="trf")

            e2sb = []
            for g in range(NG):
                e2c = e2pool.tile([D, GW], fp8, tag="e2c")
                nc.sync.dma_start(e2c[:], e2t[:, g * GW:(g + 1) * GW])
                e2sb.append(e2c)
                if g == 0:
                    for s in range(4):
                        nc.sync.dma_start(
                            trsb[32 * s:32 * s + 4, :], trhs[4 * s:4 * s + 4, :]
                        )

            for it in range(n_it):
                icols = slice(it * 128, (it + 1) * 128)
                w8 = e1sb[0:126, icols]
                acc = {}
                for is_pos in (True, False):
                    if any(s[2] == is_pos for g in range(NG) for s in group_segs(g)):
                        t = accpool.tile([128, ACCW], bf16, tag="acc", name=f"acc_{it}_{int(is_pos)}")
                        nc.vector.memset(t[:], -BIG)
                        acc[is_pos] = t
                chaincol = redpool.tile([128, 2], f32, tag="chaincol")
                chain_used = {True: False, False: False}

                def drain_group(g, ps):
                    for lo, hi, is_pos in group_segs(g):
                        ll, lh = lo - g * GW, hi - g * GW
                        wseg = lh - ll
                        ci = 0 if is_pos else 1
                        if wseg < MIN_COPY:
                            # direct f32 chained reduce of the sliver
                            nc.vector.tensor_scalar(
                                out=trf[:, 0:wseg],
                                in0=ps[:, ll:lh],
                                scalar1=(chaincol[:, ci:ci + 1]
                                         if chain_used[is_pos] else -BIG),
                                scalar2=None,
                                op0=AOT.max,
                                op1=AOT.max,
                                accum_out=chaincol[:, ci:ci + 1],
                            )
                            chain_used[is_pos] = True
                            continue
                        chunks, leftover = even_chunks(wseg)
                        cpb = cppool.tile([128, 2048], bf16, tag="cpb")
                        we = wseg - leftover
                        nc.scalar.copy(cpb[:, 0:we], ps[:, ll:ll + we])
                        for (cpos, cw) in chunks:
                            nc.vector.tensor_tensor(
                                acc[is_pos][:, 0:cw],
                                acc[is_pos][:, 0:cw],
                                cpb[:, cpos:cpos + cw],
                                op=AOT.max,
                            )
                        if leftover:
                            nc.vector.tensor_scalar(
                                out=trf[:, 0:leftover],
                                in0=ps[:, lh - leftover:lh],
                                scalar1=(chaincol[:, ci:ci + 1]
                                         if chain_used[is_pos] else -BIG),
                                scalar2=None,
                                op0=AOT.max,
                                op1=AOT.max,
                                accum_out=chaincol[:, ci:ci + 1],
                            )
                            chain_used[is_pos] = True

                for pair in range(NG // 2):
                    gA, gB = 2 * pair, 2 * pair + 1
                    pss = {}
                    for g in (gA, gB):
                        pss[g] = pspool.tile([128, GW], f32, tag="ps", name=f"ps_{it}_{g}")
                    # bias tails FIRST (start accumulation), 4-way row-packed
                    for g in (gA, gB):
                        for s in range(GW // 512):
                            j0 = g * GW + s * 512
                            nc.tensor.matmul(
                                pss[g][:, s * 512:(s + 1) * 512],
                                twsb[32 * s:32 * s + 4, icols],
                                trsb[32 * s:32 * s + 4, j0:j0 + 512],
                                start=True,
                                stop=False,
                                tile_position=(32 * s, 0),
                            )
                    # fp8 mains LAST (close accumulation -> unblock drain)
                    for g in (gA, gB):
                        for s in range(GW // 512):
                            nc.tensor.matmul(
                                pss[g][:, s * 512:(s + 1) * 512],
                                w8,
                                e2sb[g][0:126, s * 512:(s + 1) * 512],
                                start=False,
                                stop=True,
                            )
                        drain_group(g, pss[g])

                # finals: fold accumulator + chained sliver into output
                for is_pos in acc:
                    col = (0 if is_pos else n_it) + it
                    ci = 0 if is_pos else 1
                    nc.vector.tensor_scalar(
                        out=trf[:, 0:ACCW],
                        in0=acc[is_pos][:, :],
                        scalar1=(chaincol[:, ci:ci + 1]
                                 if chain_used[is_pos] else -BIG),
                        scalar2=None,
                        op0=AOT.max,
                        op1=AOT.max,
                        accum_out=outsb[:, col:col + 1],
                    )
            nc.sync.dma_start(outp[:], outsb[:])
    nc.compile()
    return nc


def _host_prep(emb1, emb2, target):
    tpos = target == 1
    k = int(tpos.sum())
    perm = np.concatenate([np.nonzero(tpos)[0], np.nonzero(~tpos)[0]])
    e2s = emb2[perm]
    e2d = e2s.astype(np.float64)
    b = (e2d * e2d).sum(1) - (2.0 * EPS) * e2d.sum(1)
    sgn = np.ones(B, dtype=np.float64)
    sgn[k:] = -1.0
    bsig = (b * sgn).astype(np.float32)
    e2sig = e2s * sgn[:, None].astype(np.float32)

    e1p = emb1[tpos]                       # [k, D] pos anchors
    e1d = e1p.astype(np.float64)
    a = (e1d * e1d).sum(1) + (2.0 * EPS) * e1d.sum(1) + D * EPS * EPS

    n_it = min(k // 1024, 8)
    ndev = n_it * 1024
    e1dev = e1p[:ndev]

    e1m2t = np.ascontiguousarray((-2.0 * e1dev).T)          # [D, ndev] f32
    e2sigt = np.ascontiguousarray(e2sig.T)                  # [D, B] f32
    e1t8 = e1m2t.astype(ml_dtypes.float8_e4m3)
    e2t8 = e2sigt.astype(ml_dtypes.float8_e4m3)
    e1tb = e1m2t.astype(ml_dtypes.bfloat16)                 # tails use bf16
    e2tb = e2sigt.astype(ml_dtypes.bfloat16)
    bhi = bsig.astype(ml_dtypes.bfloat16)
    blo = (bsig - bhi.astype(np.float32)).astype(ml_dtypes.bfloat16)
    # K=4 tail operands; on device row 4s+r lands at partition 32s+r so the
    # four 512-wide sub-tiles of a group can row-pack on the PE array.
    tailw = np.zeros((16, ndev), dtype=ml_dtypes.bfloat16)
    trhs = np.zeros((16, B), dtype=ml_dtypes.bfloat16)
    one = np.ones(ndev, dtype=ml_dtypes.bfloat16)
    for s in range(4):
        tailw[4 * s + 0] = e1tb[126]
        tailw[4 * s + 1] = e1tb[127]
        tailw[4 * s + 2] = one
        tailw[4 * s + 3] = one
        trhs[4 * s + 0] = e2tb[126]
        trhs[4 * s + 1] = e2tb[127]
        trhs[4 * s + 2] = bhi
        trhs[4 * s + 3] = blo
    return k, n_it, a, e1p, e1t8, e2t8, tailw, trhs


def _host_remainder(e1rem, emb2, target):
    """Exact f64 pos_max/neg_min contribution of the remainder anchors."""
    e1d = e1rem.astype(np.float64)
    e2d = emb2.astype(np.float64)
    sq = (
        (e1d * e1d).sum(1)[:, None]
        + (e2d * e2d).sum(1)[None, :]
        - 2.0 * (e1d @ e2d.T)
        + 2.0 * EPS * (e1d.sum(1)[:, None] - e2d.sum(1)[None, :])
        + D * EPS * EPS
    )
    dist = np.sqrt(np.clip(sq, 0.0, None))
    pos = target == 1
    pos_max = np.where(pos[None, :], dist, -np.inf).max(1)
    neg_min = np.where(~pos[None, :], dist, np.inf).min(1)
    return np.clip(pos_max - neg_min + MARGIN, 0.0, None).sum()


def _numpy_fallback(emb1, emb2, target):
    # exact reference recomputation in numpy (degenerate target mixes)
    e1 = emb1.astype(np.float64)
    e2 = emb2.astype(np.float64)
    sq = (
        (e1 * e1).sum(1)[:, None]
        + (e2 * e2).sum(1)[None, :]
        - 2.0 * (e1 @ e2.T)
        + 2.0 * EPS * (e1.sum(1)[:, None] - e2.sum(1)[None, :])
        + D * EPS * EPS
    )
    dist = np.sqrt(np.clip(sq, 0.0, None))
    pos = target == 1
    neg = target == 0
    pos_max = np.where(pos[None, :], dist, -np.inf).max(1)
    neg_min = np.where(neg[None, :], dist, np.inf).min(1)
    per = np.maximum(pos_max - neg_min + MARGIN, 0.0)
    w = pos.astype(np.float64)
    return np.float32((per * w).sum() / w.sum())


def kernel(emb1, emb2, target):
    global LAST_RESULTS
    emb1 = np.asarray(emb1, dtype=np.float32)
    emb2 = np.asarray(emb2, dtype=np.float32)
    target = np.asarray(target)
    assert emb1.shape == (B, D) and emb2.shape == (B, D)

    k = int((target == 1).sum())
    if k < 1024 or k == B:
        return _numpy_fallback(emb1, emb2, target)

    k, n_it, a, e1p, e1t8, e2t8, tailw, trhs = _host_prep(emb1, emb2, target)
    ndev = n_it * 1024
    SH = n_it * 128

    nc = _programs.get((n_it, k))
    if nc is None:
        nc = _build_program(n_it, k)
        _programs[(n_it, k)] = nc

    from concourse.bass_utils import run_bass_kernel_spmd

    in_maps = [
        {
            "e1t": np.ascontiguousarray(e1t8[:, c * SH:(c + 1) * SH]),
            "e2t": e2t8,
            "tailw": np.ascontiguousarray(tailw[:, c * SH:(c + 1) * SH]),
            "trhs": trhs,
        }
        for c in range(NCORES)
    ]
    res = run_bass_kernel_spmd(nc, in_maps, core_ids=list(range(NCORES)))
    LAST_RESULTS = res

    Mp = np.concatenate(
        [np.asarray(res.results[c]["out"])[:, :n_it].T.reshape(-1) for c in range(NCORES)]
    )
    Mn = np.concatenate(
        [np.asarray(res.results[c]["out"])[:, n_it:].T.reshape(-1) for c in range(NCORES)]
    )

    adev = a[:ndev]
    pos2 = np.clip(adev + Mp.astype(np.float64), 0.0, None)
    neg2 = np.clip(adev - Mn.astype(np.float64), 0.0, None)  # min v = -max(-v)
    per = np.clip(np.sqrt(pos2) - np.sqrt(neg2) + MARGIN, 0.0, None)
    total = per.sum()
    if ndev < k:
        total += _host_remainder(e1p[ndev:], emb2, target)
    return np.float32(total / k)


# revision 10
# speedup vs baseline: 1.5676x; 1.0194x over previous
"""BatchHardTripletLoss kernel for 8 Trainium2 NeuronCores.

Math (matches the jax reference):
  dist2[i,j] = |e1_i|^2 + |e2_j|^2 - 2 e1.e2 + 2*eps*(s1_i - s2_j) + D*eps^2
             = a[i] + v[i,j],   v[i,j] = b[j] - 2<e1_i, e2_j>
  pos_max[i] = sqrt(clip(a[i] + max_{j in pos} v[i,j], 0))
  neg_min[i] = sqrt(clip(a[i] + min_{j in neg} v[i,j], 0))
  loss = mean over POS anchors of relu(pos_max - neg_min + margin)

Key structural points (vs the 106.7us first-cut kernel):
  * Only rows with target[i]==1 are computed (the loss ignores the
    rest): k//1024*1024 rows on device, the remainder exactly on host.
  * Neg-class e2 columns (and their bias) are sign-flipped so both
    classes are MAX reductions.
  * Mains run in fp8 (e4m3): the PE streams 2 columns/cycle, so a
    512-col matmul issues every ~216ns even at the cold (K=4/8 HAM)
    clock.  The K=4 bf16 tails carry dims 126/127 + the bias in
    bf16 hi+lo.  Verified end-to-end fp8 rel err ~6.6e-4 (tol 2e-2).
  * The bias TAILS run FIRST (start=True) and the mains LAST: the
    drain of a group unblocks as soon as its mains finish, which
    overlaps the scheduler's weight-batched [tails A,B][mains A,B]
    PE order instead of serializing PE-phase -> drain-phase.
  * Drain: the Act engine copies each PSUM group to SBUF bf16
    (1 elem/cyc); the DVE folds the copies in-place into a
    [128,1024] bf16 accumulator per (i-tile, class) with stock
    TENSOR_TENSOR max ops at the packed 2x rate, then one 1x
    TENSOR_SCALAR accumulate-max per (i-tile, class) reduces the
    accumulator into the output, chained with any direct-reduced
    boundary slivers via its per-partition scalar operand.

Host: pos-first column sort, f64 row stats, bf16 hi/lo bias split,
transposes, fp8/bf16 casts, final sqrt/margin/mean + exact f64
remainder rows.
"""

import os
import sys

for _p in ("/opt/trn_rl_repo",):
    if _p not in sys.path:
        sys.path.insert(0, _p)

import numpy as np
import ml_dtypes

EPS = 1e-6
MARGIN = 0.2
B = 8192
D = 128
NCORES = 8
GW = 2048             # candidate-group width = 4 PSUM banks
NG = B // GW
PSUM_BUFS = 2
BIG = 1.0e30
MIN_COPY = 512        # segments narrower than this reduce directly from PSUM
ACCW = 1024           # bf16 accumulator width per (i-tile, class)

_programs = {}
LAST_RESULTS = None   # BassKernelResults of the most recent run (for profiling)


def _build_program(n_it: int, k: int):
    """Bass program for one core.

    n_it: i-tiles (of 128 anchors) per core.
    k: number of positive candidate columns (boundary between the pos
       range [0,k) and the sign-flipped neg range [k,B)).
    """
    import concourse.bacc as bacc
    import concourse.tile as tile
    from concourse import mybir

    f32 = mybir.dt.float32
    bf16 = mybir.dt.bfloat16
    fp8 = mybir.dt.float8e4
    AOT = mybir.AluOpType

    SH = n_it * 128

    nc = bacc.Bacc(None)
    e1t = nc.declare_dram_parameter("e1t", [D, SH], fp8, isOutput=False)
    e2t = nc.declare_dram_parameter("e2t", [D, B], fp8, isOutput=False)
    tailw = nc.declare_dram_parameter("tailw", [16, SH], bf16, isOutput=False)
    trhs = nc.declare_dram_parameter("trhs", [16, B], bf16, isOutput=False)
    outp = nc.declare_dram_parameter("out", [128, 2 * n_it], f32, isOutput=True)

    # per-group reduction segments: (lo, hi, is_pos) in global column coords
    def group_segs(g):
        glo, ghi = g * GW, (g + 1) * GW
        segs = []
        if glo < k:
            segs.append((glo, min(ghi, k), True))
        if ghi > k:
            segs.append((max(glo, k), ghi, False))
        return segs

    def even_chunks(w):
        """Split [0,w) into even-width chunks <= ACCW; odd leftover col
        is returned separately (handled by the direct f32 path)."""
        out = []
        pos = 0
        we = (w // 2) * 2
        while pos < we:
            cw = min(ACCW, we - pos)
            out.append((pos, cw))
            pos += cw
        return out, (w - we)   # chunks, n leftover cols (0 or 1)

    with tile.TileContext(nc) as tc:
        with (
            tc.tile_pool(name="const", bufs=1) as cpool,
            tc.tile_pool(name="e2p", bufs=NG) as e2pool,
            tc.tile_pool(name="ps", bufs=PSUM_BUFS, space="PSUM") as pspool,
            tc.tile_pool(name="cp", bufs=3) as cppool,
            tc.tile_pool(name="acc", bufs=4) as accpool,
            tc.tile_pool(name="red", bufs=2) as redpool,
        ):
            e1sb = cpool.tile([D, SH], fp8, tag="e1sb")
            nc.sync.dma_start(e1sb[:], e1t[:])
            twsb = cpool.tile([128, SH], bf16, tag="twsb")
            trsb = cpool.tile([128, B], bf16, tag="trsb")
            for s in range(4):
                nc.sync.dma_start(twsb[32 * s:32 * s + 4, :], tailw[4 * s:4 * s + 4, :])
                nc.sync.dma_start(trsb[32 * s:32 * s + 4, :], trhs[4 * s:4 * s + 4, :])
            outsb = cpool.tile([128, 2 * n_it], f32, tag="outsb")
            trf = cpool.tile([128, 2048], f32, tag="trf")

            e2sb = []
            for g in range(NG):
                e2c = e2pool.tile([D, GW], fp8, tag="e2c")
                nc.sync.dma_start(e2c[:], e2t[:, g * GW:(g + 1) * GW])
                e2sb.append(e2c)

            for it in range(n_it):
                icols = slice(it * 128, (it + 1) * 128)
                w8 = e1sb[0:126, icols]
                acc = {}
                for is_pos in (True, False):
                    if any(s[2] == is_pos for g in range(NG) for s in group_segs(g)):
                        t = accpool.tile([128, ACCW], bf16, tag="acc", name=f"acc_{it}_{int(is_pos)}")
                        nc.gpsimd.memset(t[:], -BIG)
                        acc[is_pos] = t
                chaincol = redpool.tile([128, 2], f32, tag="chaincol")
                chain_used = {True: False, False: False}

                def drain_group(g, ps):
                    for lo, hi, is_pos in group_segs(g):
                        ll, lh = lo - g * GW, hi - g * GW
                        wseg = lh - ll
                        ci = 0 if is_pos else 1
                        if wseg < MIN_COPY:
                            # direct f32 chained reduce of the sliver
                            nc.vector.tensor_scalar(
                                out=trf[:, 0:wseg],
                                in0=ps[:, ll:lh],
                                scalar1=(chaincol[:, ci:ci + 1]
                                         if chain_used[is_pos] else -BIG),
                                scalar2=None,
                                op0=AOT.max,
                                op1=AOT.max,
                                accum_out=chaincol[:, ci:ci + 1],
                            )
                            chain_used[is_pos] = True
                            continue
                        chunks, leftover = even_chunks(wseg)
                        cpb = cppool.tile([128, 2048], bf16, tag="cpb")
                        we = wseg - leftover
                        nc.scalar.copy(cpb[:, 0:we], ps[:, ll:ll + we])
                        for (cpos, cw) in chunks:
                            nc.vector.tensor_tensor(
                                acc[is_pos][:, 0:cw],
                                acc[is_pos][:, 0:cw],
                                cpb[:, cpos:cpos + cw],
                                op=AOT.max,
                            )
                        if leftover:
                            nc.vector.tensor_scalar(
                                out=trf[:, 0:leftover],
                                in0=ps[:, lh - leftover:lh],
                                scalar1=(chaincol[:, ci:ci + 1]
                                         if chain_used[is_pos] else -BIG),
                                scalar2=None,
                                op0=AOT.max,
                                op1=AOT.max,
                                accum_out=chaincol[:, ci:ci + 1],
                            )
                            chain_used[is_pos] = True

                for pair in range(NG // 2):
                    gA, gB = 2 * pair, 2 * pair + 1
                    pss = {}
                    for g in (gA, gB):
                        pss[g] = pspool.tile([128, GW], f32, tag="ps", name=f"ps_{it}_{g}")
                    # bias tails FIRST (start accumulation), 4-way row-packed
                    for g in (gA, gB):
                        for s in range(GW // 512):
                            j0 = g * GW + s * 512
                            nc.tensor.matmul(
                                pss[g][:, s * 512:(s + 1) * 512],
                                twsb[32 * s:32 * s + 4, icols],
                                trsb[32 * s:32 * s + 4, j0:j0 + 512],
                                start=True,
                                stop=False,
                                tile_position=(32 * s, 0),
                            )
                    # fp8 mains LAST (close accumulation -> unblock drain)
                    for g in (gA, gB):
                        for s in range(GW // 512):
                            nc.tensor.matmul(
                                pss[g][:, s * 512:(s + 1) * 512],
                                w8,
                                e2sb[g][0:126, s * 512:(s + 1) * 512],
                                start=False,
                                stop=True,
                            )
                        drain_group(g, pss[g])

                # finals: fold accumulator + chained sliver into output
                for is_pos in acc:
                    col = (0 if is_pos else n_it) + it
                    ci = 0 if is_pos else 1
                    nc.vector.tensor_scalar(
                        out=trf[:, 0:ACCW],
                        in0=acc[is_pos][:, :],
                        scalar1=(chaincol[:, ci:ci + 1]
                                 if chain_used[is_pos] else -BIG),
                        scalar2=None,
                        op0=AOT.max,
                        op1=AOT.max,
                        accum_out=outsb[:, col:col + 1],
                    )
            nc.sync.dma_start(outp[:], outsb[:])
    nc.compile()
    return nc


def _host_prep(emb1, emb2, target):
    tpos = target == 1
    k = int(tpos.sum())
    perm = np.concatenate([np.nonzero(tpos)[0], np.nonzero(~tpos)[0]])
    e2s = emb2[perm]
    e2d = e2s.astype(np.float64)
    b = (e2d * e2d).sum(1) - (2.0 * EPS) * e2d.sum(1)
    sgn = np.ones(B, dtype=np.float64)
    sgn[k:] = -1.0
    bsig = (b * sgn).astype(np.float32)
    e2sig = e2s * sgn[:, None].astype(np.float32)

    e1p = emb1[tpos]                       # [k, D] pos anchors
    e1d = e1p.astype(np.float64)
    a = (e1d * e1d).sum(1) + (2.0 * EPS) * e1d.sum(1) + D * EPS * EPS

    n_it = min(k // 1024, 8)
    ndev = n_it * 1024
    e1dev = e1p[:ndev]

    e1m2t = np.ascontiguousarray((-2.0 * e1dev).T)          # [D, ndev] f32
    e2sigt = np.ascontiguousarray(e2sig.T)                  # [D, B] f32
    e1t8 = e1m2t.astype(ml_dtypes.float8_e4m3)
    e2t8 = e2sigt.astype(ml_dtypes.float8_e4m3)
    e1tb = e1m2t.astype(ml_dtypes.bfloat16)                 # tails use bf16
    e2tb = e2sigt.astype(ml_dtypes.bfloat16)
    bhi = bsig.astype(ml_dtypes.bfloat16)
    blo = (bsig - bhi.astype(np.float32)).astype(ml_dtypes.bfloat16)
    # K=4 tail operands; on device row 4s+r lands at partition 32s+r so the
    # four 512-wide sub-tiles of a group can row-pack on the PE array.
    tailw = np.zeros((16, ndev), dtype=ml_dtypes.bfloat16)
    trhs = np.zeros((16, B), dtype=ml_dtypes.bfloat16)
    one = np.ones(ndev, dtype=ml_dtypes.bfloat16)
    for s in range(4):
        tailw[4 * s + 0] = e1tb[126]
        tailw[4 * s + 1] = e1tb[127]
        tailw[4 * s + 2] = one
        tailw[4 * s + 3] = one
        trhs[4 * s + 0] = e2tb[126]
        trhs[4 * s + 1] = e2tb[127]
        trhs[4 * s + 2] = bhi
        trhs[4 * s + 3] = blo
    return k, n_it, a, e1p, e1t8, e2t8, tailw, trhs


def _host_remainder(e1rem, emb2, target):
    """Exact f64 pos_max/neg_min contribution of the remainder anchors."""
    e1d = e1rem.astype(np.float64)
    e2d = emb2.astype(np.float64)
    sq = (
        (e1d * e1d).sum(1)[:, None]
        + (e2d * e2d).sum(1)[None, :]
        - 2.0 * (e1d @ e2d.T)
        + 2.0 * EPS * (e1d.sum(1)[:, None] - e2d.sum(1)[None, :])
        + D * EPS * EPS
    )
    dist = np.sqrt(np.clip(sq, 0.0, None))
    pos = target == 1
    pos_max = np.where(pos[None, :], dist, -np.inf).max(1)
    neg_min = np.where(~pos[None, :], dist, np.inf).min(1)
    return np.clip(pos_max - neg_min + MARGIN, 0.0, None).sum()


def _numpy_fallback(emb1, emb2, target):
    # exact reference recomputation in numpy (degenerate target mixes)
    e1 = emb1.astype(np.float64)
    e2 = emb2.astype(np.float64)
    sq = (
        (e1 * e1).sum(1)[:, None]
        + (e2 * e2).sum(1)[None, :]
        - 2.0 * (e1 @ e2.T)
        + 2.0 * EPS * (e1.sum(1)[:, None] - e2.sum(1)[None, :])
        + D * EPS * EPS
    )
    dist = np.sqrt(np.clip(sq, 0.0, None))
    pos = target == 1
    neg = target == 0
    pos_max = np.where(pos[None, :], dist, -np.inf).max(1)
    neg_min = np.where(neg[None, :], dist, np.inf).min(1)
    per = np.maximum(pos_max - neg_min + MARGIN, 0.0)
    w = pos.astype(np.float64)
    return np.float32((per * w).sum() / w.sum())


def kernel(emb1, emb2, target):
    global LAST_RESULTS
    emb1 = np.asarray(emb1, dtype=np.float32)
    emb2 = np.asarray(emb2, dtype=np.float32)
    target = np.asarray(target)
    assert emb1.shape == (B, D) and emb2.shape == (B, D)

    k = int((target == 1).sum())
    if k < 1024 or k == B:
        return _numpy_fallback(emb1, emb2, target)

    k, n_it, a, e1p, e1t8, e2t8, tailw, trhs = _host_prep(emb1, emb2, target)
    ndev = n_it * 1024
    SH = n_it * 128

    nc = _programs.get((n_it, k))
    if nc is None:
        nc = _build_program(n_it, k)
        _programs[(n_it, k)] = nc

    from concourse.bass_utils import run_bass_kernel_spmd

    in_maps = [
        {
            "e1t": np.ascontiguousarray(e1t8[:, c * SH:(c + 1) * SH]),
            "e2t": e2t8,
            "tailw": np.ascontiguousarray(tailw[:, c * SH:(c + 1) * SH]),
            "trhs": trhs,
        }
        for c in range(NCORES)
    ]
    res = run_bass_kernel_spmd(nc, in_maps, core_ids=list(range(NCORES)))
    LAST_RESULTS = res

    Mp = np.concatenate(
        [np.asarray(res.results[c]["out"])[:, :n_it].T.reshape(-1) for c in range(NCORES)]
    )
    Mn = np.concatenate(
        [np.asarray(res.results[c]["out"])[:, n_it:].T.reshape(-1) for c in range(NCORES)]
    )

    adev = a[:ndev]
    pos2 = np.clip(adev + Mp.astype(np.float64), 0.0, None)
    neg2 = np.clip(adev - Mn.astype(np.float64), 0.0, None)  # min v = -max(-v)
    per = np.clip(np.sqrt(pos2) - np.sqrt(neg2) + MARGIN, 0.0, None)
    total = per.sum()
    if ndev < k:
        total += _host_remainder(e1p[ndev:], emb2, target)
    return np.float32(total / k)


# revision 16
# speedup vs baseline: 1.6299x; 1.0397x over previous
"""BatchHardTripletLoss kernel for 8 Trainium2 NeuronCores.

Math (matches the jax reference):
  dist2[i,j] = |e1_i|^2 + |e2_j|^2 - 2 e1.e2 + 2*eps*(s1_i - s2_j) + D*eps^2
             = a[i] + v[i,j],   v[i,j] = b[j] - 2<e1_i, e2_j>
  pos_max[i] = sqrt(clip(a[i] + max_{j in pos} v[i,j], 0))
  neg_min[i] = sqrt(clip(a[i] + min_{j in neg} v[i,j], 0))
  loss = mean over POS anchors of relu(pos_max - neg_min + margin)

Key structural points (vs the 106.7us first-cut kernel):
  * Only rows with target[i]==1 are computed (the loss ignores the
    rest): k//1024*1024 rows on device, the remainder exactly on host.
  * Neg-class e2 columns (and their bias) are sign-flipped so both
    classes are MAX reductions.
  * Mains run in fp8 (e4m3): the PE streams 2 columns/cycle, so a
    512-col matmul issues every ~216ns even at the cold (K=4/8 HAM)
    clock.  The K=4 bf16 tails carry dims 126/127 + the bias in
    bf16 hi+lo.  Verified end-to-end fp8 rel err ~6.6e-4 (tol 2e-2).
  * The bias TAILS run FIRST (start=True) and the mains LAST: the
    drain of a group unblocks as soon as its mains finish, which
    overlaps the scheduler's weight-batched [tails A,B][mains A,B]
    PE order instead of serializing PE-phase -> drain-phase.
  * Drain: the Act engine copies each PSUM group to SBUF bf16
    (1 elem/cyc); the DVE folds the copies in-place into a
    [128,1024] bf16 accumulator per (i-tile, class) with stock
    TENSOR_TENSOR max ops at the packed 2x rate, then one 1x
    TENSOR_SCALAR accumulate-max per (i-tile, class) reduces the
    accumulator into the output, chained with any direct-reduced
    boundary slivers via its per-partition scalar operand.

Host: pos-first column sort, f64 row stats, bf16 hi/lo bias split,
transposes, fp8/bf16 casts, final sqrt/margin/mean + exact f64
remainder rows.
"""

import os
import sys

for _p in ("/opt/trn_rl_repo",):
    if _p not in sys.path:
        sys.path.insert(0, _p)

import numpy as np
import ml_dtypes

EPS = 1e-6
MARGIN = 0.2
B = 8192
D = 128
NCORES = 8
GW = 2048             # candidate-group width = 4 PSUM banks
NG = B // GW
PSUM_BUFS = 2
BIG = 1.0e30
MIN_COPY = 512        # segments narrower than this reduce directly from PSUM
ACCW = 1024           # bf16 accumulator width per (i-tile, class)

_programs = {}
LAST_RESULTS = None   # BassKernelResults of the most recent run (for profiling)


def _build_program(n_it: int, k: int):
    """Bass program for one core.

    n_it: i-tiles (of 128 anchors) per core.
    k: number of positive candidate columns (boundary between the pos
       range [0,k) and the sign-flipped neg range [k,B)).
    """
    import concourse.bacc as bacc
    import concourse.tile as tile
    from concourse import mybir

    f32 = mybir.dt.float32
    bf16 = mybir.dt.bfloat16
    fp8 = mybir.dt.float8e4
    AOT = mybir.AluOpType

    SH = n_it * 128

    nc = bacc.Bacc(None)
    e1t = nc.declare_dram_parameter("e1t", [D, SH], fp8, isOutput=False)
    e2t = nc.declare_dram_parameter("e2t", [D, B], fp8, isOutput=False)
    tails = nc.declare_dram_parameter("tails", [16, SH + B], bf16, isOutput=False)
    outp = nc.declare_dram_parameter("out", [128, 2 * n_it], f32, isOutput=True)

    # per-group reduction segments: (lo, hi, is_pos) in global column coords
    def group_segs(g):
        glo, ghi = g * GW, (g + 1) * GW
        segs = []
        if glo < k:
            segs.append((glo, min(ghi, k), True))
        if ghi > k:
            segs.append((max(glo, k), ghi, False))
        return segs

    def even_chunks(w):
        """Split [0,w) into even-width chunks <= ACCW; odd leftover col
        is returned separately (handled by the direct f32 path)."""
        out = []
        pos = 0
        we = (w // 2) * 2
        while pos < we:
            cw = min(ACCW, we - pos)
            out.append((pos, cw))
            pos += cw
        return out, (w - we)   # chunks, n leftover cols (0 or 1)

    with tile.TileContext(nc) as tc:
        with (
            tc.tile_pool(name="const", bufs=1) as cpool,
            tc.tile_pool(name="e2p", bufs=NG) as e2pool,
            tc.tile_pool(name="ps", bufs=PSUM_BUFS, space="PSUM") as pspool,
            tc.tile_pool(name="cp", bufs=3) as cppool,
            tc.tile_pool(name="acc", bufs=4) as accpool,
            tc.tile_pool(name="red", bufs=2) as redpool,
        ):
            # merged tail operands: strip s on partitions 32s..32s+3,
            # cols [0:SH] = lhsT (e1 dims 126/127 + ones), [SH:] = rhs
            tlsb = cpool.tile([128, SH + B], bf16, tag="tlsb")
            for s in range(4):
                nc.sync.dma_start(tlsb[32 * s:32 * s + 4, :], tails[4 * s:4 * s + 4, :])
            e1sb = cpool.tile([D, SH], fp8, tag="e1sb")
            nc.sync.dma_start(e1sb[:], e1t[:])
            outsb = cpool.tile([128, 2 * n_it], f32, tag="outsb")
            trf = cpool.tile([128, 2048], f32, tag="trf")

            e2sb = []
            for g in range(NG):
                e2c = e2pool.tile([D, GW], fp8, tag="e2c")
                nc.scalar.dma_start(e2c[:], e2t[:, g * GW:(g + 1) * GW])
                e2sb.append(e2c)

            for it in range(n_it):
                icols = slice(it * 128, (it + 1) * 128)
                w8 = e1sb[0:126, icols]
                acc = {}
                for is_pos in (True, False):
                    if any(s[2] == is_pos for g in range(NG) for s in group_segs(g)):
                        t = accpool.tile([128, ACCW], bf16, tag="acc", name=f"acc_{it}_{int(is_pos)}")
                        nc.gpsimd.memset(t[:], -BIG)
                        acc[is_pos] = t
                chaincol = redpool.tile([128, 2], f32, tag="chaincol")
                chain_used = {True: False, False: False}

                def drain_group(g, ps):
                    for lo, hi, is_pos in group_segs(g):
                        ll, lh = lo - g * GW, hi - g * GW
                        wseg = lh - ll
                        ci = 0 if is_pos else 1
                        if wseg < MIN_COPY:
                            # direct f32 chained reduce of the sliver
                            nc.vector.tensor_scalar(
                                out=trf[:, 0:wseg],
                                in0=ps[:, ll:lh],
                                scalar1=(chaincol[:, ci:ci + 1]
                                         if chain_used[is_pos] else -BIG),
                                scalar2=None,
                                op0=AOT.max,
                                op1=AOT.max,
                                accum_out=chaincol[:, ci:ci + 1],
                            )
                            chain_used[is_pos] = True
                            continue
                        chunks, leftover = even_chunks(wseg)
                        cpb = cppool.tile([128, 2048], bf16, tag="cpb")
                        we = wseg - leftover
                        nc.scalar.copy(cpb[:, 0:we], ps[:, ll:ll + we])
                        for (cpos, cw) in chunks:
                            nc.vector.tensor_tensor(
                                acc[is_pos][:, 0:cw],
                                acc[is_pos][:, 0:cw],
                                cpb[:, cpos:cpos + cw],
                                op=AOT.max,
                            )
                        if leftover:
                            nc.vector.tensor_scalar(
                                out=trf[:, 0:leftover],
                                in0=ps[:, lh - leftover:lh],
                                scalar1=(chaincol[:, ci:ci + 1]
                                         if chain_used[is_pos] else -BIG),
                                scalar2=None,
                                op0=AOT.max,
                                op1=AOT.max,
                                accum_out=chaincol[:, ci:ci + 1],
                            )
                            chain_used[is_pos] = True

                for pair in range(NG // 2):
                    gA, gB = 2 * pair, 2 * pair + 1
                    pss = {}
                    for g in (gA, gB):
                        pss[g] = pspool.tile([128, GW], f32, tag="ps", name=f"ps_{it}_{g}")
                    # bias tails FIRST (start accumulation), 4-way row-packed
                    for g in (gA, gB):
                        for s in range(GW // 512):
                            j0 = SH + g * GW + s * 512
                            nc.tensor.matmul(
                                pss[g][:, s * 512:(s + 1) * 512],
                                tlsb[32 * s:32 * s + 4, icols],
                                tlsb[32 * s:32 * s + 4, j0:j0 + 512],
                                start=True,
                                stop=False,
                                tile_position=(32 * s, 0),
                            )
                    # fp8 mains LAST (close accumulation -> unblock drain)
                    for g in (gA, gB):
                        for s in range(GW // 512):
                            nc.tensor.matmul(
                                pss[g][:, s * 512:(s + 1) * 512],
                                w8,
                                e2sb[g][0:126, s * 512:(s + 1) * 512],
                                start=False,
                                stop=True,
                            )
                        drain_group(g, pss[g])

                # finals: fold accumulator + chained sliver into output
                for is_pos in acc:
                    col = (0 if is_pos else n_it) + it
                    ci = 0 if is_pos else 1
                    nc.vector.tensor_scalar(
                        out=trf[:, 0:ACCW],
                        in0=acc[is_pos][:, :],
                        scalar1=(chaincol[:, ci:ci + 1]
                                 if chain_used[is_pos] else -BIG),
                        scalar2=None,
                        op0=AOT.max,
                        op1=AOT.max,
                        accum_out=outsb[:, col:col + 1],
                    )
            nc.sync.dma_start(outp[:], outsb[:])
    nc.compile()
    return nc


def _host_prep(emb1, emb2, target):
    tpos = target == 1
    k = int(tpos.sum())
    perm = np.concatenate([np.nonzero(tpos)[0], np.nonzero(~tpos)[0]])
    e2s = emb2[perm]
    e2d = e2s.astype(np.float64)
    b = (e2d * e2d).sum(1) - (2.0 * EPS) * e2d.sum(1)
    sgn = np.ones(B, dtype=np.float64)
    sgn[k:] = -1.0
    bsig = (b * sgn).astype(np.float32)
    e2sig = e2s * sgn[:, None].astype(np.float32)

    e1p = emb1[tpos]                       # [k, D] pos anchors
    e1d = e1p.astype(np.float64)
    a = (e1d * e1d).sum(1) + (2.0 * EPS) * e1d.sum(1) + D * EPS * EPS

    n_it = min(k // 1024, 8)
    ndev = n_it * 1024
    e1dev = e1p[:ndev]

    e1m2t = np.ascontiguousarray((-2.0 * e1dev).T)          # [D, ndev] f32
    e2sigt = np.ascontiguousarray(e2sig.T)                  # [D, B] f32
    e1t8 = e1m2t.astype(ml_dtypes.float8_e4m3)
    e2t8 = e2sigt.astype(ml_dtypes.float8_e4m3)
    e1tb = e1m2t.astype(ml_dtypes.bfloat16)                 # tails use bf16
    e2tb = e2sigt.astype(ml_dtypes.bfloat16)
    bhi = bsig.astype(ml_dtypes.bfloat16)
    blo = (bsig - bhi.astype(np.float32)).astype(ml_dtypes.bfloat16)
    # K=4 tail operands; on device row 4s+r lands at partition 32s+r so the
    # four 512-wide sub-tiles of a group can row-pack on the PE array.
    # Per-core layout: cols [0:SH] = lhsT (per-core slice), [SH:] = rhs.
    SH = n_it * 128
    tailw = np.zeros((16, ndev), dtype=ml_dtypes.bfloat16)
    trhs = np.zeros((16, B), dtype=ml_dtypes.bfloat16)
    one = np.ones(ndev, dtype=ml_dtypes.bfloat16)
    for s in range(4):
        tailw[4 * s + 0] = e1tb[126]
        tailw[4 * s + 1] = e1tb[127]
        tailw[4 * s + 2] = one
        tailw[4 * s + 3] = one
        trhs[4 * s + 0] = e2tb[126]
        trhs[4 * s + 1] = e2tb[127]
        trhs[4 * s + 2] = bhi
        trhs[4 * s + 3] = blo
    tails = [
        np.concatenate([tailw[:, c * SH:(c + 1) * SH], trhs], axis=1)
        for c in range(NCORES)
    ]
    return k, n_it, a, e1p, e1t8, e2t8, tails


def _host_remainder(e1rem, emb2, target):
    """Exact f64 pos_max/neg_min contribution of the remainder anchors."""
    e1d = e1rem.astype(np.float64)
    e2d = emb2.astype(np.float64)
    sq = (
        (e1d * e1d).sum(1)[:, None]
        + (e2d * e2d).sum(1)[None, :]
        - 2.0 * (e1d @ e2d.T)
        + 2.0 * EPS * (e1d.sum(1)[:, None] - e2d.sum(1)[None, :])
        + D * EPS * EPS
    )
    dist = np.sqrt(np.clip(sq, 0.0, None))
    pos = target == 1
    pos_max = np.where(pos[None, :], dist, -np.inf).max(1)
    neg_min = np.where(~pos[None, :], dist, np.inf).min(1)
    return np.clip(pos_max - neg_min + MARGIN, 0.0, None).sum()


def _numpy_fallback(emb1, emb2, target):
    # exact reference recomputation in numpy (degenerate target mixes)
    e1 = emb1.astype(np.float64)
    e2 = emb2.astype(np.float64)
    sq = (
        (e1 * e1).sum(1)[:, None]
        + (e2 * e2).sum(1)[None, :]
        - 2.0 * (e1 @ e2.T)
        + 2.0 * EPS * (e1.sum(1)[:, None] - e2.sum(1)[None, :])
        + D * EPS * EPS
    )
    dist = np.sqrt(np.clip(sq, 0.0, None))
    pos = target == 1
    neg = target == 0
    pos_max = np.where(pos[None, :], dist, -np.inf).max(1)
    neg_min = np.where(neg[None, :], dist, np.inf).min(1)
    per = np.maximum(pos_max - neg_min + MARGIN, 0.0)
    w = pos.astype(np.float64)
    return np.float32((per * w).sum() / w.sum())


def kernel(emb1, emb2, target):
    global LAST_RESULTS
    emb1 = np.asarray(emb1, dtype=np.float32)
    emb2 = np.asarray(emb2, dtype=np.float32)
    target = np.asarray(target)
    assert emb1.shape == (B, D) and emb2.shape == (B, D)

    k = int((target == 1).sum())
    if k < 1024 or k == B:
        return _numpy_fallback(emb1, emb2, target)

    k, n_it, a, e1p, e1t8, e2t8, tails = _host_prep(emb1, emb2, target)
    ndev = n_it * 1024
    SH = n_it * 128

    nc = _programs.get((n_it, k))
    if nc is None:
        nc = _build_program(n_it, k)
        _programs[(n_it, k)] = nc

    from concourse.bass_utils import run_bass_kernel_spmd

    in_maps = [
        {
            "e1t": np.ascontiguousarray(e1t8[:, c * SH:(c + 1) * SH]),
            "e2t": e2t8,
            "tails": np.ascontiguousarray(tails[c]),
        }
        for c in range(NCORES)
    ]
    res = run_bass_kernel_spmd(nc, in_maps, core_ids=list(range(NCORES)))
    LAST_RESULTS = res

    Mp = np.concatenate(
        [np.asarray(res.results[c]["out"])[:, :n_it].T.reshape(-1) for c in range(NCORES)]
    )
    Mn = np.concatenate(
        [np.asarray(res.results[c]["out"])[:, n_it:].T.reshape(-1) for c in range(NCORES)]
    )

    adev = a[:ndev]
    pos2 = np.clip(adev + Mp.astype(np.float64), 0.0, None)
    neg2 = np.clip(adev - Mn.astype(np.float64), 0.0, None)  # min v = -max(-v)
    per = np.clip(np.sqrt(pos2) - np.sqrt(neg2) + MARGIN, 0.0, None)
    total = per.sum()
    if ndev < k:
        total += _host_remainder(e1p[ndev:], emb2, target)
    return np.float32(total / k)


# revision 22
# speedup vs baseline: 1.8965x; 1.1636x over previous
"""BatchHardTripletLoss kernel for 8 Trainium2 NeuronCores.

Math (matches the jax reference):
  dist2[i,j] = |e1_i|^2 + |e2_j|^2 - 2 e1.e2 + 2*eps*(s1_i - s2_j) + D*eps^2
             = a[i] + v[i,j],   v[i,j] = b[j] - 2<e1_i, e2_j>
  pos_max[i] = sqrt(clip(a[i] + max_{j in pos} v[i,j], 0))
  neg_min[i] = sqrt(clip(a[i] + min_{j in neg} v[i,j], 0))
  loss = mean over POS anchors of relu(pos_max - neg_min + margin)

Key structural points (vs the 106.7us first-cut kernel):
  * Only rows with target[i]==1 are computed (the loss ignores the
    rest): k//1024*1024 rows on device, the remainder exactly on host.
  * Neg-class e2 columns (and their bias) are sign-flipped so both
    classes are MAX reductions.
  * Mains run in fp8 (e4m3): the PE streams 2 columns/cycle, so a
    512-col matmul issues every ~216ns even at the cold (K=4/8 HAM)
    clock.  The K=4 bf16 tails carry dims 126/127 + the bias in
    bf16 hi+lo.  Verified end-to-end fp8 rel err ~6.6e-4 (tol 2e-2).
  * The bias TAILS run FIRST (start=True) and the mains LAST: the
    drain of a group unblocks as soon as its mains finish, which
    overlaps the scheduler's weight-batched [tails A,B][mains A,B]
    PE order instead of serializing PE-phase -> drain-phase.
  * Drain: the Act engine copies each PSUM group to SBUF bf16
    (1 elem/cyc); the DVE folds the copies in-place into a
    [128,1024] bf16 accumulator per (i-tile, class) with stock
    TENSOR_TENSOR max ops at the packed 2x rate, then one 1x
    TENSOR_SCALAR accumulate-max per (i-tile, class) reduces the
    accumulator into the output, chained with any direct-reduced
    boundary slivers via its per-partition scalar operand.

Host: pos-first column sort, f64 row stats, bf16 hi/lo bias split,
transposes, fp8/bf16 casts, final sqrt/margin/mean + exact f64
remainder rows.
"""

import os
import sys

for _p in ("/opt/trn_rl_repo",):
    if _p not in sys.path:
        sys.path.insert(0, _p)

import numpy as np
import ml_dtypes

EPS = 1e-6
MARGIN = 0.2
B = 8192
D = 128
NCORES = 8
GW = 2048             # candidate-group width = 4 PSUM banks
NG = B // GW
PSUM_BUFS = 2
BIG = 1.0e30
MIN_COPY = 512        # segments narrower than this reduce directly from PSUM
ACCW = 1024           # bf16 accumulator width per (i-tile, class)

_programs = {}
LAST_RESULTS = None   # BassKernelResults of the most recent run (for profiling)


def _build_program(n_it: int, k: int):
    """Bass program for one core.

    n_it: i-tiles (of 128 anchors) per core.
    k: number of positive candidate columns (boundary between the pos
       range [0,k) and the sign-flipped neg range [k,B)).
    """
    import concourse.bacc as bacc
    import concourse.tile as tile
    from concourse import mybir

    f32 = mybir.dt.float32
    bf16 = mybir.dt.bfloat16
    fp8 = mybir.dt.float8e4
    AOT = mybir.AluOpType

    SH = n_it * 128

    nc = bacc.Bacc(None)
    e1t = nc.declare_dram_parameter("e1t", [D, SH], fp8, isOutput=False)
    e2t = nc.declare_dram_parameter("e2t", [D, B], fp8, isOutput=False)
    tails = nc.declare_dram_parameter("tails", [8, SH + B], bf16, isOutput=False)
    outp = nc.declare_dram_parameter("out", [128, 2 * n_it], f32, isOutput=True)

    # per-group reduction segments: (lo, hi, is_pos) in global column coords
    def group_segs(g):
        glo, ghi = g * GW, (g + 1) * GW
        segs = []
        if glo < k:
            segs.append((glo, min(ghi, k), True))
        if ghi > k:
            segs.append((max(glo, k), ghi, False))
        return segs

    def even_chunks(w):
        """Split [0,w) into even-width chunks <= ACCW; odd leftover col
        is returned separately (handled by the direct f32 path)."""
        out = []
        pos = 0
        we = (w // 2) * 2
        while pos < we:
            cw = min(ACCW, we - pos)
            out.append((pos, cw))
            pos += cw
        return out, (w - we)   # chunks, n leftover cols (0 or 1)

    with tile.TileContext(nc) as tc:
        with (
            tc.tile_pool(name="const", bufs=1) as cpool,
            tc.tile_pool(name="e2p", bufs=NG) as e2pool,
            tc.tile_pool(name="ps", bufs=PSUM_BUFS, space="PSUM") as pspool,
            tc.tile_pool(name="cp", bufs=3) as cppool,
            tc.tile_pool(name="acc", bufs=4) as accpool,
            tc.tile_pool(name="red", bufs=2) as redpool,
        ):
            # merged bias-tail operands: strip s on partitions 32s..32s+1,
            # cols [0:SH] = lhsT (ones), [SH:] = rhs (bias hi/lo).  Split
            # each strip across both HWDGE queues: the destination spans
            # only 2 partitions, so per-partition write rate is the wall.
            tlsb = cpool.tile([128, SH + B], bf16, tag="tlsb")
            half = (SH + B) // 2
            for s in range(4):
                nc.sync.dma_start(
                    tlsb[32 * s:32 * s + 2, 0:half], tails[2 * s:2 * s + 2, 0:half]
                )
                nc.scalar.dma_start(
                    tlsb[32 * s:32 * s + 2, half:], tails[2 * s:2 * s + 2, half:]
                )
            e1sb = cpool.tile([D, SH], fp8, tag="e1sb")
            nc.sync.dma_start(e1sb[:], e1t[:])
            outsb = cpool.tile([128, 2 * n_it], f32, tag="outsb")
            trf = cpool.tile([128, 2048], f32, tag="trf")

            e2sb = []
            for g in range(NG):
                e2c = e2pool.tile([D, GW], fp8, tag="e2c")
                nc.scalar.dma_start(e2c[:], e2t[:, g * GW:(g + 1) * GW])
                e2sb.append(e2c)

            for it in range(n_it):
                icols = slice(it * 128, (it + 1) * 128)
                w8 = e1sb[:, icols]
                acc = {}
                for is_pos in (True, False):
                    if any(s[2] == is_pos for g in range(NG) for s in group_segs(g)):
                        t = accpool.tile([128, ACCW], bf16, tag="acc", name=f"acc_{it}_{int(is_pos)}")
                        nc.gpsimd.memset(t[:], -BIG)
                        acc[is_pos] = t
                chaincol = redpool.tile([128, 2], f32, tag="chaincol")
                chain_used = {True: False, False: False}

                def drain_group(g, ps):
                    for lo, hi, is_pos in group_segs(g):
                        ll, lh = lo - g * GW, hi - g * GW
                        wseg = lh - ll
                        ci = 0 if is_pos else 1
                        if wseg < MIN_COPY:
                            # direct f32 chained reduce of the sliver
                            nc.vector.tensor_scalar(
                                out=trf[:, 0:wseg],
                                in0=ps[:, ll:lh],
                                scalar1=(chaincol[:, ci:ci + 1]
                                         if chain_used[is_pos] else -BIG),
                                scalar2=None,
                                op0=AOT.max,
                                op1=AOT.max,
                                accum_out=chaincol[:, ci:ci + 1],
                            )
                            chain_used[is_pos] = True
                            continue
                        chunks, leftover = even_chunks(wseg)
                        cpb = cppool.tile([128, 2048], bf16, tag="cpb")
                        we = wseg - leftover
                        nc.scalar.copy(cpb[:, 0:we], ps[:, ll:ll + we])
                        for (cpos, cw) in chunks:
                            nc.vector.tensor_tensor(
                                acc[is_pos][:, 0:cw],
                                acc[is_pos][:, 0:cw],
                                cpb[:, cpos:cpos + cw],
                                op=AOT.max,
                            )
                        if leftover:
                            nc.vector.tensor_scalar(
                                out=trf[:, 0:leftover],
                                in0=ps[:, lh - leftover:lh],
                                scalar1=(chaincol[:, ci:ci + 1]
                                         if chain_used[is_pos] else -BIG),
                                scalar2=None,
                                op0=AOT.max,
                                op1=AOT.max,
                                accum_out=chaincol[:, ci:ci + 1],
                            )
                            chain_used[is_pos] = True

                for pair in range(NG // 2):
                    gA, gB = 2 * pair, 2 * pair + 1
                    pss = {}
                    for g in (gA, gB):
                        pss[g] = pspool.tile([128, GW], f32, tag="ps", name=f"ps_{it}_{g}")
                    # bias tails FIRST (start accumulation), 4-way row-packed
                    for g in (gA, gB):
                        for s in range(GW // 512):
                            j0 = SH + g * GW + s * 512
                            nc.tensor.matmul(
                                pss[g][:, s * 512:(s + 1) * 512],
                                tlsb[32 * s:32 * s + 2, icols],
                                tlsb[32 * s:32 * s + 2, j0:j0 + 512],
                                start=True,
                                stop=False,
                                tile_position=(32 * s, 0),
                            )
                    # fp8 K=128 mains LAST (close accumulation -> unblock drain)
                    for g in (gA, gB):
                        for s in range(GW // 512):
                            nc.tensor.matmul(
                                pss[g][:, s * 512:(s + 1) * 512],
                                w8,
                                e2sb[g][:, s * 512:(s + 1) * 512],
                                start=False,
                                stop=True,
                            )
                        drain_group(g, pss[g])

                # finals: fold accumulator + chained sliver into output
                for is_pos in acc:
                    col = (0 if is_pos else n_it) + it
                    ci = 0 if is_pos else 1
                    nc.vector.tensor_scalar(
                        out=trf[:, 0:ACCW],
                        in0=acc[is_pos][:, :],
                        scalar1=(chaincol[:, ci:ci + 1]
                                 if chain_used[is_pos] else -BIG),
                        scalar2=None,
                        op0=AOT.max,
                        op1=AOT.max,
                        accum_out=outsb[:, col:col + 1],
                    )
            nc.sync.dma_start(outp[:], outsb[:])
    nc.compile()
    return nc


def _host_prep(emb1, emb2, target):
    tpos = target == 1
    k = int(tpos.sum())
    perm = np.concatenate([np.nonzero(tpos)[0], np.nonzero(~tpos)[0]])
    e2s = emb2[perm]
    e2d = e2s.astype(np.float64)
    b = (e2d * e2d).sum(1) - (2.0 * EPS) * e2d.sum(1)
    sgn = np.ones(B, dtype=np.float64)
    sgn[k:] = -1.0
    bsig = (b * sgn).astype(np.float32)
    e2sig = e2s * sgn[:, None].astype(np.float32)

    e1p = emb1[tpos]                       # [k, D] pos anchors
    e1d = e1p.astype(np.float64)
    a = (e1d * e1d).sum(1) + (2.0 * EPS) * e1d.sum(1) + D * EPS * EPS

    n_it = min(k // 1024, 8)
    ndev = n_it * 1024
    e1dev = e1p[:ndev]

    e1m2t = np.ascontiguousarray((-2.0 * e1dev).T)          # [D, ndev] f32
    e2sigt = np.ascontiguousarray(e2sig.T)                  # [D, B] f32
    e1t8 = e1m2t.astype(ml_dtypes.float8_e4m3)
    e2t8 = e2sigt.astype(ml_dtypes.float8_e4m3)
    e1tb = e1m2t.astype(ml_dtypes.bfloat16)                 # tails use bf16
    e2tb = e2sigt.astype(ml_dtypes.bfloat16)
    bhi = bsig.astype(ml_dtypes.bfloat16)
    blo = (bsig - bhi.astype(np.float32)).astype(ml_dtypes.bfloat16)
    # K=2 bias-tail operands; on device row 2s+r lands at partition 32s+r
    # so the four 512-wide sub-tiles of a group row-pack on the PE array.
    # Per-core layout: cols [0:SH] = lhsT (ones), [SH:] = rhs (bias).
    SH = n_it * 128
    trhs = np.zeros((8, B), dtype=ml_dtypes.bfloat16)
    for s in range(4):
        trhs[2 * s + 0] = bhi
        trhs[2 * s + 1] = blo
    onesw = np.ones((8, SH), dtype=ml_dtypes.bfloat16)
    tails = np.concatenate([onesw, trhs], axis=1)
    return k, n_it, a, e1p, e1t8, e2t8, tails


def _host_remainder(e1rem, emb2, target):
    """Exact f64 pos_max/neg_min contribution of the remainder anchors."""
    e1d = e1rem.astype(np.float64)
    e2d = emb2.astype(np.float64)
    sq = (
        (e1d * e1d).sum(1)[:, None]
        + (e2d * e2d).sum(1)[None, :]
        - 2.0 * (e1d @ e2d.T)
        + 2.0 * EPS * (e1d.sum(1)[:, None] - e2d.sum(1)[None, :])
        + D * EPS * EPS
    )
    dist = np.sqrt(np.clip(sq, 0.0, None))
    pos = target == 1
    pos_max = np.where(pos[None, :], dist, -np.inf).max(1)
    neg_min = np.where(~pos[None, :], dist, np.inf).min(1)
    return np.clip(pos_max - neg_min + MARGIN, 0.0, None).sum()


def _numpy_fallback(emb1, emb2, target):
    # exact reference recomputation in numpy (degenerate target mixes)
    e1 = emb1.astype(np.float64)
    e2 = emb2.astype(np.float64)
    sq = (
        (e1 * e1).sum(1)[:, None]
        + (e2 * e2).sum(1)[None, :]
        - 2.0 * (e1 @ e2.T)
        + 2.0 * EPS * (e1.sum(1)[:, None] - e2.sum(1)[None, :])
        + D * EPS * EPS
    )
    dist = np.sqrt(np.clip(sq, 0.0, None))
    pos = target == 1
    neg = target == 0
    pos_max = np.where(pos[None, :], dist, -np.inf).max(1)
    neg_min = np.where(neg[None, :], dist, np.inf).min(1)
    per = np.maximum(pos_max - neg_min + MARGIN, 0.0)
    w = pos.astype(np.float64)
    return np.float32((per * w).sum() / w.sum())


def kernel(emb1, emb2, target):
    global LAST_RESULTS
    emb1 = np.asarray(emb1, dtype=np.float32)
    emb2 = np.asarray(emb2, dtype=np.float32)
    target = np.asarray(target)
    assert emb1.shape == (B, D) and emb2.shape == (B, D)

    k = int((target == 1).sum())
    if k < 1024 or k == B:
        return _numpy_fallback(emb1, emb2, target)

    k, n_it, a, e1p, e1t8, e2t8, tails = _host_prep(emb1, emb2, target)
    ndev = n_it * 1024
    SH = n_it * 128

    nc = _programs.get((n_it, k))
    if nc is None:
        nc = _build_program(n_it, k)
        _programs[(n_it, k)] = nc

    from concourse.bass_utils import run_bass_kernel_spmd

    in_maps = [
        {
            "e1t": np.ascontiguousarray(e1t8[:, c * SH:(c + 1) * SH]),
            "e2t": e2t8,
            "tails": tails,
        }
        for c in range(NCORES)
    ]
    res = run_bass_kernel_spmd(nc, in_maps, core_ids=list(range(NCORES)))
    LAST_RESULTS = res

    Mp = np.concatenate(
        [np.asarray(res.results[c]["out"])[:, :n_it].T.reshape(-1) for c in range(NCORES)]
    )
    Mn = np.concatenate(
        [np.asarray(res.results[c]["out"])[:, n_it:].T.reshape(-1) for c in range(NCORES)]
    )

    adev = a[:ndev]
    pos2 = np.clip(adev + Mp.astype(np.float64), 0.0, None)
    neg2 = np.clip(adev - Mn.astype(np.float64), 0.0, None)  # min v = -max(-v)
    per = np.clip(np.sqrt(pos2) - np.sqrt(neg2) + MARGIN, 0.0, None)
    total = per.sum()
    if ndev < k:
        total += _host_remainder(e1p[ndev:], emb2, target)
    return np.float32(total / k)
